# revision 1
# baseline (speedup 1.0000x reference)
"""CoaT factorized-attention block kernel for Trainium2, 8 NeuronCores.

Sharding: data-parallel over batch B=8 -> one batch element per core.

Per-core pipeline (all attention-path tensors in head-aligned [96, *] layout):
  P1  (per 128-token chunk): k,v = x @ Wk/Wv (natural layout, PE);
      E = exp(k) (ACT); kv_aug[h] += E_h^T @ [v_h | 1] (PE, PSUM-resident
      accumulator across chunks); v^T via PE transpose -> vt[h] [96, 3136].
  P2  kv[h] = kv_aug[:, :96] * scale / kv_aug[:, 96]  (DVE).
  P3  cls column: q_cls, factor_att, proj (tiny).
  P4  (per 8-image-row group g of 448 cols): per head: depthwise conv as
      per-tap diagonal matmuls accumulating in PSUM (PE); q^T chunk (PE);
      factor = kv_h^T-free matmul (PE); att = factor + q*conv (DVE);
      then proj: out^T[e, cols] = sum_h pw96_h @ att_h (PE) + bias (ACT).

Matmul inputs bf16 (fp32 PSUM accumulation); ~0.5% rel error vs fp32 ref.
"""
import numpy as np
import ml_dtypes

B, N, C = 8, 3137, 768
NH, CH = 8, 96            # heads, channels per head
H = W = 56
HW = H * W                # 3136 == N - 1
NK = 7                    # contraction k-tiles of 128 over 896 (768 + bias row + pad)
KPAD = NK * 128           # 896
HEAD_KS = [3, 3, 5, 5, 5, 7, 7, 7]
SCALE = CH ** -0.5
GROUPS = 7                # image-row groups of 8 rows = 448 cols each
GC = 8 * W                # 448

bf16 = ml_dtypes.bfloat16


def _head_taps(k):
    p = k // 2
    return [(0, 0)] + [(dy, dx) for dy in range(-p, p + 1)
                       for dx in range(-p, p + 1) if (dy, dx) != (0, 0)]


TAPS = [_head_taps(k) for k in HEAD_KS]
TAP_OFF = np.cumsum([0] + [len(t) for t in TAPS]).tolist()  # offsets into dg
NTAP = TAP_OFF[-1]  # 240

_PROG = None


def _build_program():
    import concourse.bass as bass
    from concourse import bacc
    import concourse.mybir as mybir
    import concourse.tile as tile
    from concourse.masks import make_identity
    from contextlib import ExitStack

    fp32 = mybir.dt.float32
    bf = mybir.dt.bfloat16

    nc = bacc.Bacc("TRN2", target_bir_lowering=False, debug=False, num_devices=8)

    xT_d = nc.dram_tensor("xT", [KPAD, N], bf, kind="ExternalInput")
    wq_d = nc.dram_tensor("wq", [KPAD, C], bf, kind="ExternalInput")
    wk_d = nc.dram_tensor("wk", [KPAD, C], bf, kind="ExternalInput")
    wv_d = nc.dram_tensor("wv", [KPAD, C], bf, kind="ExternalInput")
    pw_d = nc.dram_tensor("pw96", [CH, NH, C], bf, kind="ExternalInput")
    pb_d = nc.dram_tensor("pb2", [128, 6], fp32, kind="ExternalInput")
    cb_d = nc.dram_tensor("cb96", [CH, NH], fp32, kind="ExternalInput")
    dg_d = nc.dram_tensor("dg", [CH, NTAP, CH], bf, kind="ExternalInput")
    out_d = nc.dram_tensor("outT", [C, N], fp32, kind="ExternalOutput")

    xT_r = xT_d[:, :].rearrange("(t p) n -> p t n", p=128)  # [128, 7, N]
    wq_r = wq_d[:, :].rearrange("(t p) c -> p t c", p=128)
    wk_r = wk_d[:, :].rearrange("(t p) c -> p t c", p=128)
    wv_r = wv_d[:, :].rearrange("(t p) c -> p t c", p=128)

    # token chunks for pass 1: cls + 24 full + tail(64), img-aligned after cls
    chunks = [(0, 1)] + [(1 + 128 * t, 128) for t in range(24)] + [(3073, 64)]

    with tile.TileContext(nc) as tc, ExitStack() as ctx:
        const = ctx.enter_context(tc.tile_pool(name="const", bufs=1))
        wq_sb = const.tile([128, NK, C], bf)
        nc.sync.dma_start(wq_sb, wq_r)
        pw_sb = const.tile([CH, NH, C], bf)
        nc.sync.dma_start(pw_sb, pw_d[:, :, :])
        pb_sb = const.tile([128, 6], fp32)
        nc.sync.dma_start(pb_sb, pb_d[:, :])
        cb_sb = const.tile([CH, NH], fp32)
        nc.sync.dma_start(cb_sb, cb_d[:, :])
        dg_sb = const.tile([CH, NTAP, CH], bf)
        nc.sync.dma_start(dg_sb, dg_d[:, :, :])
        ident = const.tile([128, 128], bf)
        make_identity(nc, ident)
        ones = const.tile([128, 1], bf)
        nc.vector.memset(ones, 1.0)
        vt = [const.tile([CH, HW], bf, tag=f"vt{h}", name=f"vt{h}") for h in range(NH)]
        kv_sb = const.tile([CH, NH, CH], bf)
        r_sb = const.tile([CH, NH], fp32)

        # ---------------- pass 1: k, v, E, kv accumulation, v transposes
        with tc.tile_pool(name="p1w", bufs=1) as p1w, \
             tc.tile_pool(name="p1roll", bufs=3) as p1roll, \
             tc.tile_pool(name="p1ps", bufs=2, space="PSUM") as p1ps, \
             tc.tile_pool(name="kvps", bufs=1, space="PSUM") as kvps:
            wk_sb = p1w.tile([128, NK, C], bf)
            nc.sync.dma_start(wk_sb, wk_r)
            wv_sb = p1w.tile([128, NK, C], bf)
            nc.sync.dma_start(wv_sb, wv_r)

            kv_ps = [kvps.tile([CH, 4, CH + 1], fp32, tag=f"kv{i}", name=f"kvps{i}") for i in range(2)]

            for ci, (n0, sz) in enumerate(chunks):
                first, last = ci == 0, ci == len(chunks) - 1
                xh = p1roll.tile([128, NK, 128], bf, tag="xh")
                nc.sync.dma_start(xh[:, :, :sz], xT_r[:, :, n0:n0 + sz])

                v_sb = p1roll.tile([128, C], bf, tag="v")
                e_sb = p1roll.tile([128, C], bf, tag="e")
                for half in range(2):
                    c0 = half * 384
                    pv = p1ps.tile([128, 384], fp32, tag="pv")
                    pk = p1ps.tile([128, 384], fp32, tag="pk")
                    for k in range(NK):
                        nc.tensor.matmul(pv[:sz], xh[:, k, :sz], wv_sb[:, k, c0:c0 + 384],
                                         start=(k == 0), stop=(k == NK - 1))
                    for k in range(NK):
                        nc.tensor.matmul(pk[:sz], xh[:, k, :sz], wk_sb[:, k, c0:c0 + 384],
                                         start=(k == 0), stop=(k == NK - 1))
                    nc.scalar.copy(v_sb[:sz, c0:c0 + 384], pv[:sz])
                    nc.scalar.activation(e_sb[:sz, c0:c0 + 384], pk[:sz],
                                         mybir.ActivationFunctionType.Exp)

                for h in range(NH):
                    kvp = kv_ps[h // 4]
                    nc.tensor.matmul(kvp[:, h % 4, 0:CH],
                                     e_sb[:sz, h * CH:(h + 1) * CH],
                                     v_sb[:sz, h * CH:(h + 1) * CH],
                                     start=first, stop=last, skip_group_check=True)
                    nc.tensor.matmul(kvp[:, h % 4, CH:CH + 1],
                                     e_sb[:sz, h * CH:(h + 1) * CH],
                                     ones[:sz],
                                     start=first, stop=last, skip_group_check=True)

                if not first:  # transpose image tokens into vt[h]
                    j0 = n0 - 1
                    for h in range(NH):
                        tp = p1ps.tile([CH, 128], bf, tag="tp")
                        nc.tensor.transpose(tp[:, :sz],
                                            v_sb[:sz, h * CH:(h + 1) * CH],
                                            ident[:sz, :sz])
                        if h % 2 == 0:
                            nc.vector.tensor_copy(vt[h][:, j0:j0 + sz], tp[:, :sz])
                        else:
                            nc.scalar.copy(vt[h][:, j0:j0 + sz], tp[:, :sz])

            # finalize kv: kv = kv_aug[:, :96] * (1/den) * scale
            for h in range(NH):
                kvp = kv_ps[h // 4]
                nc.vector.reciprocal(r_sb[:, h:h + 1], kvp[:, h % 4, CH:CH + 1])
                nc.vector.tensor_scalar(kv_sb[:, h, :], kvp[:, h % 4, 0:CH],
                                        r_sb[:, h:h + 1], float(SCALE),
                                        op0=mybir.AluOpType.mult,
                                        op1=mybir.AluOpType.mult)

        # ---------------- pass 2: per-group conv + q + factor + att + proj
        with tc.tile_pool(name="p4roll", bufs=3) as p4roll, \
             tc.tile_pool(name="p4att", bufs=2) as p4att, \
             tc.tile_pool(name="p4xg", bufs=2) as p4xg, \
             tc.tile_pool(name="p4ps", bufs=2, space="PSUM") as p4ps:

            # cls column (token 0): factor_att only, crpe = 0
            xc = p4xg.tile([128, NK, GC], bf, tag="xg")
            nc.sync.dma_start(xc[:, :, 0:1], xT_r[:, :, 0:1])
            pqc = p4ps.tile([CH, NH], fp32, tag="pq")
            for h in range(NH):
                for k in range(NK):
                    nc.tensor.matmul(pqc[:, h:h + 1], wq_sb[:, k, h * CH:(h + 1) * CH],
                                     xc[:, k, 0:1], start=(k == 0), stop=(k == NK - 1),
                                     skip_group_check=True)
            qtc = p4roll.tile([CH, NH], bf, tag="qtc")
            nc.scalar.copy(qtc, pqc)
            pfc = p4ps.tile([CH, NH], fp32, tag="pf")
            for h in range(NH):
                nc.tensor.matmul(pfc[:, h:h + 1], kv_sb[:, h, :], qtc[:, h:h + 1],
                                 start=True, stop=True, skip_group_check=True)
            atc = p4roll.tile([CH, NH], bf, tag="atc")
            nc.scalar.copy(atc, pfc)
            poc = p4ps.tile([128, GC], fp32, tag="po")
            for e in range(6):
                for h in range(NH):
                    nc.tensor.matmul(poc[:, e:e + 1], pw_sb[:, h, e * 128:(e + 1) * 128],
                                     atc[:, h:h + 1], start=(h == 0), stop=(h == NH - 1),
                                     skip_group_check=True)
            ocs = p4roll.tile([128, 6], fp32, tag="ocs")
            for e in range(6):
                nc.scalar.activation(ocs[:, e:e + 1], poc[:, e:e + 1],
                                     mybir.ActivationFunctionType.Identity,
                                     bias=pb_sb[:, e:e + 1])
                nc.sync.dma_start(out_d[e * 128:(e + 1) * 128, 0:1], ocs[:, e:e + 1])

            # main grouped loop
            vt3 = [vt[h].rearrange("p (y x) -> p y x", y=H) for h in range(NH)]
            for g in range(GROUPS):
                gy0, gy1 = g * 8, g * 8 + 8
                n0 = 1 + g * GC  # token index of first col in group
                xg = p4xg.tile([128, NK, GC], bf, tag="xg")
                nc.sync.dma_start(xg, xT_r[:, :, n0:n0 + GC])

                att = p4att.tile([CH, NH, GC], bf, tag="att")
                for h in range(NH):
                    # conv: per-tap diagonal matmuls accumulating in psum
                    pcv = p4ps.tile([CH, 8, W], fp32, tag="pcv")
                    taps = TAPS[h]
                    t_base = TAP_OFF[h]
                    # which taps actually hit this group
                    live = []
                    for t, (dy, dx) in enumerate(taps):
                        y0 = max(gy0, -dy)
                        y1 = min(gy1, H - max(0, dy))
                        if y1 > y0:
                            live.append((t, dy, dx, y0, y1))
                    assert live[0][0] == 0  # (0,0) full-range first
                    for li, (t, dy, dx, y0, y1) in enumerate(live):
                        x0 = max(0, -dx)
                        x1 = W - max(0, dx)
                        out_ap = pcv[:, y0 - gy0:y1 - gy0, x0:x1]
                        in_ap = vt3[h][:, y0 + dy:y1 + dy, x0 + dx:x1 + dx]
                        nc.tensor.matmul(out_ap, dg_sb[:, t_base + t, :], in_ap,
                                         start=(li == 0), stop=(li == len(live) - 1),
                                         skip_group_check=True)
                    cv = p4roll.tile([CH, GC], bf, tag="cv")
                    nc.scalar.activation(cv, pcv.rearrange("p a b -> p (a b)"),
                                         mybir.ActivationFunctionType.Identity,
                                         bias=cb_sb[:, h:h + 1])

                    # q^T chunk for this head
                    pq = p4ps.tile([CH, GC], fp32, tag="pq")
                    for k in range(NK):
                        nc.tensor.matmul(pq, wq_sb[:, k, h * CH:(h + 1) * CH],
                                         xg[:, k, :], start=(k == 0), stop=(k == NK - 1))
                    qt = p4roll.tile([CH, GC], bf, tag="qt")
                    nc.scalar.copy(qt, pq)

                    # factor_att
                    pf = p4ps.tile([CH, GC], fp32, tag="pf")
                    nc.tensor.matmul(pf, kv_sb[:, h, :], qt, start=True, stop=True)

                    # att = factor + q * conv
                    ev = p4roll.tile([CH, GC], bf, tag="ev")
                    nc.vector.tensor_tensor(ev, qt, cv, op=mybir.AluOpType.mult)
                    nc.vector.scalar_tensor_tensor(att[:, h, :], pf, 1.0, ev,
                                                   op0=mybir.AluOpType.mult,
                                                   op1=mybir.AluOpType.add)

                # proj for this group of columns
                for e in range(6):
                    po = p4ps.tile([128, GC], fp32, tag="po")
                    for h in range(NH):
                        nc.tensor.matmul(po, pw_sb[:, h, e * 128:(e + 1) * 128],
                                         att[:, h, :], start=(h == 0), stop=(h == NH - 1))
                    osb = p4roll.tile([128, GC], fp32, tag="osb")
                    nc.scalar.activation(osb, po,
                                         mybir.ActivationFunctionType.Identity,
                                         bias=pb_sb[:, e:e + 1])
                    nc.sync.dma_start(out_d[e * 128:(e + 1) * 128, n0:n0 + GC], osb)

    nc.compile()
    return nc


def _get_program():
    global _PROG
    if _PROG is None:
        _PROG = _build_program()
    return _PROG


def _host_prep(x, qkv_w, qkv_b, proj_w, proj_b,
               conv3_w, conv3_b, conv5_w, conv5_b, conv7_w, conv7_b):
    """Build per-core input dicts (shared weight tensors prepped once)."""
    qkv_w = np.asarray(qkv_w, np.float32)
    qkv_b = np.asarray(qkv_b, np.float32)
    proj_w = np.asarray(proj_w, np.float32)
    proj_b = np.asarray(proj_b, np.float32)

    def wslab(sl):
        w = np.zeros((KPAD, C), np.float32)
        w[0:C] = qkv_w[sl].T
        w[C] = qkv_b[sl]
        return w.astype(bf16)

    wq = wslab(slice(0, C))
    wk = wslab(slice(C, 2 * C))
    wv = wslab(slice(2 * C, 3 * C))

    pw96 = np.ascontiguousarray(
        proj_w.T.reshape(NH, CH, C).transpose(1, 0, 2)).astype(bf16)
    pb2 = np.ascontiguousarray(proj_b.reshape(6, 128).T).astype(np.float32)

    conv_w = [np.asarray(w, np.float32) for w in (conv3_w, conv5_w, conv7_w)]
    conv_b = [np.asarray(b, np.float32) for b in (conv3_b, conv5_b, conv7_b)]
    grp_of_head = [0, 0, 1, 1, 1, 2, 2, 2]
    head_in_grp = [0, 1, 0, 1, 2, 0, 1, 2]

    cb96 = np.zeros((CH, NH), np.float32)
    dg = np.zeros((CH, NTAP, CH), np.float32)
    for h in range(NH):
        k = HEAD_KS[h]
        p = k // 2
        gidx, hig = grp_of_head[h], head_in_grp[h]
        wfull = conv_w[gidx][hig * CH:(hig + 1) * CH, 0]  # [96, k, k]
        cb96[:, h] = conv_b[gidx][hig * CH:(hig + 1) * CH]
        for t, (dy, dx) in enumerate(TAPS[h]):
            np.fill_diagonal(dg[:, TAP_OFF[h] + t, :], wfull[:, dy + p, dx + p])
    dg = dg.astype(bf16)

    shared = {"wq": wq, "wk": wk, "wv": wv, "pw96": pw96, "pb2": pb2,
              "cb96": cb96, "dg": dg}

    x = np.asarray(x, np.float32)
    in_maps = []
    for b in range(B):
        xT = np.zeros((KPAD, N), np.float32)
        xT[0:C] = x[b].T
        xT[C] = 1.0
        m = dict(shared)
        m["xT"] = xT.astype(bf16)
        in_maps.append(m)
    return in_maps


def kernel(x, qkv_w, qkv_b, proj_w, proj_b,
           conv3_w, conv3_b, conv5_w, conv5_b, conv7_w, conv7_b, H, W,
           _trace=False):
    assert int(H) == 56 and int(W) == 56
    x = np.asarray(x)
    assert x.shape == (B, N, C)

    from concourse.bass_utils import run_bass_kernel_spmd
    nc = _get_program()
    in_maps = _host_prep(x, qkv_w, qkv_b, proj_w, proj_b,
                         conv3_w, conv3_b, conv5_w, conv5_b, conv7_w, conv7_b)
    res = run_bass_kernel_spmd(nc, in_maps, core_ids=list(range(B)), trace=_trace)
    out = np.stack([res.results[b]["outT"].T for b in range(B)])
    if _trace:
        kernel._last_results = res
    return out.astype(np.float32)




# revision 2
# speedup vs baseline: 1.0266x; 1.0266x over previous
"""CoaT factorized-attention block kernel for Trainium2, 8 NeuronCores.

Data-parallel over batch B=8 -> one batch element per core. All-fp16
operands (fp32 PSUM accumulation); dense 128-feature tile layout.

Per-core pipeline:
  pass1a (features 384:768 = heads 4-7, tiles 3-5) per 128-token chunk:
    k,v = x @ Wk/Wv (PE); E = exp(k) (ACT; k-bias cancels in the
    token-axis softmax); kv_aug[h] += E_h^T @ [v_h | 1] (PE, PSUM);
    v^T via PE transpose -> vt tiles (ACT copies, +v-bias).
  padded copies vtp/vtpo/vtm (DMA) for shift-FMA conv reads.
  DVE conv band 0 (rows 0:32) -- overlaps pass1b.
  pass1b: features 0:384 (heads 0-3, tiles 0-2); copies on ACT.
  kv finalize (DVE) -> KV6 block-diag stationary tiles (DMA).
  groups g=0..6 (448 tokens): q dense (PE) -> qt6 (ACT); PE conv taps
    (full-array diagonal matmuls); factor via KV6 (PE); ev = qt*cv,
    att = ev + factor (DVE); proj (PE) -> bias copy (ACT) -> DMA.
    DVE band 1 (rows 32:56) issued after group 3.
  cls token: dedicated tiny q/factor/proj path (crpe = 0).
"""
import numpy as np

B, N, C = 8, 3137, 768
NH, CH = 8, 96
H = W = 56
HW = H * W
NK = 6
NT = 6
GROUPS, GC = 7, 448
SCALE = CH ** -0.5
HEAD_KS = [3, 3, 5, 5, 5, 7, 7, 7]
TILE_KMAX = [3, 5, 5, 7, 7, 7]
fp16 = np.float16


def _taps(k):
    p = k // 2
    return [(0, 0)] + [(dy, dx) for dy in range(-p, p + 1)
                       for dx in range(-p, p + 1) if (dy, dx) != (0, 0)]


TAPS6 = [_taps(k) for k in TILE_KMAX]

# ownership: PE gets the first PE_CNT[t] taps of each tile (full-array
# diagonal matmuls); the rest run as mult(ACT or DVE) + add(DVE) over
# padded shift-copies vtp/vtpo/vtm.
PE_CNT = [9, 25, 25, 49, 12, 0]
PAD_TILES = [3, 4, 5]     # tiles whose conv reads padded copies


def pe_taps(t):
    return list(range(PE_CNT[t]))


def dve_taps(t):
    return list(range(PE_CNT[t], len(TAPS6[t])))


DVE_TILES = [t for t in range(NT) if dve_taps(t)]


PE_COL = {}
_c = 0
for _t in range(NT):
    for _ti in pe_taps(_t):
        PE_COL[(_t, _ti)] = _c
        _c += 1
NPECOL = _c
DVE_COL = {}
_c = 0
for _t in range(NT):
    for _ti in dve_taps(_t):
        DVE_COL[(_t, _ti)] = _c
        _c += 1
NDVECOL = _c

KVBLOCKS = []
for _h in range(NH):
    _r0, _r1 = 96 * _h, 96 * _h + 96
    for _tk in range(_r0 // 128, (_r1 - 1) // 128 + 1):
        for _t in range(_r0 // 128, (_r1 - 1) // 128 + 1):
            if (_tk, _t) not in KVBLOCKS:
                KVBLOCKS.append((_tk, _t))
KVIDX = {blk: i for i, blk in enumerate(KVBLOCKS)}
NKVB = len(KVBLOCKS)

HALVES = [(384, 768, range(4, 8), range(3, 6)),
          (0, 384, range(0, 4), range(0, 3))]
# pass1a: 112-token chunks (= 2 image rows) so v^T lands directly in the
# padded row-pitch-64 layout; pass1b: 128-token chunks (flat vt6).
CHUNKS_A = [(0, 1)] + [(1 + 112 * t, 112) for t in range(28)]
CHUNKS_B = [(0, 1)] + [(1 + 128 * t, 128) for t in range(24)] + [(3073, 64)]

_PROG = None


def _build_program():
    from concourse import bacc
    import concourse.mybir as mybir
    import concourse.tile as tile
    from concourse.masks import make_identity
    from contextlib import ExitStack

    f32 = mybir.dt.float32
    f16 = mybir.dt.float16
    AL = mybir.AluOpType
    AF = mybir.ActivationFunctionType

    nc = bacc.Bacc("TRN2", target_bir_lowering=False, debug=False,
                   num_devices=8)

    xT_d = nc.dram_tensor("xT", [C, N], f16, kind="ExternalInput")
    wk_d = nc.dram_tensor("wk", [128, NK, C], f16, kind="ExternalInput")
    wv_d = nc.dram_tensor("wv", [128, NK, C], f16, kind="ExternalInput")
    wq_d = nc.dram_tensor("wq6", [128, NK, C], f16, kind="ExternalInput")
    pw_d = nc.dram_tensor("pw6", [128, NT, C], f16, kind="ExternalInput")
    dg_d = nc.dram_tensor("dg6", [128, NPECOL, 128], f16,
                          kind="ExternalInput")
    ws_d = nc.dram_tensor("w6s", [128, NDVECOL], f32, kind="ExternalInput")
    cb_d = nc.dram_tensor("cb6", [128, NT], f32, kind="ExternalInput")
    bq_d = nc.dram_tensor("bq6", [128, NT], f32, kind="ExternalInput")
    pb_d = nc.dram_tensor("pb6", [128, NT], f32, kind="ExternalInput")
    bv_d = nc.dram_tensor("bv6", [128, NT], f32, kind="ExternalInput")
    bkv_d = nc.dram_tensor("bvkv", [CH, NH * CH], f16, kind="ExternalInput")
    out_d = nc.dram_tensor("outT", [C, N], f32, kind="ExternalOutput")

    xT_r = xT_d[:, :].rearrange("(t p) n -> p t n", p=128)

    with tile.TileContext(nc) as tc, ExitStack() as ctx:
        const = ctx.enter_context(tc.tile_pool(name="const", bufs=1))
        wq_sb = const.tile([128, NK, C], f16)
        pw_sb = const.tile([128, NT, C], f16)
        dg_sb = const.tile([128, NPECOL, 128], f16)
        ws_sb = const.tile([128, NDVECOL], f32)
        nc.sync.dma_start(ws_sb, ws_d[:, :])
        cb_sb = const.tile([128, NT], f32)
        nc.sync.dma_start(cb_sb, cb_d[:, :])
        bq_sb = const.tile([128, NT], f32)
        nc.sync.dma_start(bq_sb, bq_d[:, :])
        pb_sb = const.tile([128, NT], f32)
        nc.sync.dma_start(pb_sb, pb_d[:, :])
        bv_sb = const.tile([128, NT], f32)
        nc.sync.dma_start(bv_sb, bv_d[:, :])
        bkv_sb = const.tile([CH, NH * CH], f16)
        nc.sync.dma_start(bkv_sb, bkv_d[:, :])
        ident = const.tile([128, 128], f16)
        make_identity(nc, ident)
        ones = const.tile([128, 1], f16)
        nc.vector.memset(ones, 1.0)

        # v^T for PE-conv tiles 0-2 (written in pass1b, read all groups)
        vt6 = const.tile([128, 3, HW], f16)
        vt3 = vt6.rearrange("p t (y x) -> p t y x", y=H)

        kv_sb = const.tile([CH, NH, CH], f16)
        r_sb = const.tile([CH, NH], f32)
        kv6 = const.tile([128, NKVB, 128], f16)
        for i in range(NKVB):
            nc.vector.memset(kv6[:, i, :], 0.0)

        # padded shift-copies + DVE accumulators
        vtp, vtpo, vtm, acc6 = {}, {}, {}, {}
        for t in PAD_TILES:
            vtp[t] = const.tile([128, 57, 64], f16, tag=f"vtp{t}",
                                name=f"vtp{t}")
        for t in DVE_TILES:
            vtpo[t] = const.tile([128, 57, 64], f16, tag=f"vtpo{t}",
                                 name=f"vtpo{t}")
            vtm[t] = const.tile([128, 57, 64], f16, tag=f"vtm{t}",
                                name=f"vtm{t}")
            acc6[t] = const.tile([128, HW], f16, tag=f"acc{t}",
                                 name=f"acc{t}")
        cvt_pool = ctx.enter_context(tc.tile_pool(name="cvt", bufs=3))

        def _tap_src(t, dy, dx, y0, y1):
            if dx % 2 == 0:
                src, xo = vtp[t], dx
            elif dx > 0:
                src, xo = vtpo[t], dx - 1
            else:
                src, xo = vtm[t], dx + 1
            if xo >= 0:
                return src[:, 1 + y0 + dy:1 + y1 + dy, xo:xo + 56]
            o0 = (1 + y0 + dy) * 64 + xo
            return src.rearrange("p a b -> p (a b)") \
                [:, o0:o0 + (y1 - y0) * 64] \
                .rearrange("p (a b) -> p a b", b=64)[:, :, 0:56]

        def band_ops(b0, b1):
            """Off-PE conv op list for rows [b0,b1), tiles interleaved."""
            ops = []
            maxtap = max(len(dve_taps(t)) for t in DVE_TILES)
            for i in range(maxtap):
                for t in DVE_TILES:
                    taps = dve_taps(t)
                    if i >= len(taps):
                        continue
                    ti = taps[i]
                    dy, dx = TAPS6[t][ti]
                    y0 = max(b0, -dy)
                    y1 = min(b1, H - max(0, dy))
                    if y1 <= y0:
                        continue
                    ops.append((t, ti, dy, dx, y0, y1,
                                i == 0 and not pe_taps(t)))
            return ops

        def emit_conv(ops, act_budget):
            acc3 = {t: acc6[t].rearrange("p (y x) -> p y x", y=H)
                    for t in DVE_TILES}
            nact = 0
            for (t, ti, dy, dx, y0, y1, seed) in ops:
                sview = _tap_src(t, dy, dx, y0, y1)
                w = ws_sb[:, DVE_COL[(t, ti)]:DVE_COL[(t, ti)] + 1]
                dst = acc3[t][:, y0:y1, :]
                if seed:
                    # tap (0,0): full band coverage -> seed + conv bias
                    nc.vector.tensor_scalar(dst, sview, w,
                                            cb_sb[:, t:t + 1],
                                            op0=AL.mult, op1=AL.add)
                    continue
                tmp = cvt_pool.tile([128, 32, 56], f16, tag="cvt",
                                    name="cvt")
                tv = tmp[:, 0:y1 - y0, :]
                if nact < act_budget:
                    nact += 1
                    nc.scalar.mul(tv, sview, w)
                else:
                    nc.vector.tensor_scalar_mul(tv, sview, w)
                nc.vector.tensor_tensor(dst, dst, tv, op=AL.add)

        with tc.tile_pool(name="kvps", bufs=1, space="PSUM") as kvpsp:
            kv_ps = [kvpsp.tile([CH, 4, CH + 1], f32, tag=f"kv{i}",
                                name=f"kvps{i}") for i in range(2)]

            def pass1(hp):
                c0, c1, heads, tiles = HALVES[hp]
                chunks = CHUNKS_A if hp == 0 else CHUNKS_B
                with tc.tile_pool(name=f"p1w{hp}", bufs=1) as p1w, \
                     tc.tile_pool(name=f"p1r{hp}", bufs=3) as p1r, \
                     tc.tile_pool(name=f"p1ps{hp}", bufs=2,
                                  space="PSUM") as p1ps:
                    wk_sb = p1w.tile([128, NK, 384], f16, name="wk_sb")
                    nc.sync.dma_start(wk_sb, wk_d[:, :, c0:c1])
                    wv_sb = p1w.tile([128, NK, 384], f16, name="wv_sb")
                    nc.sync.dma_start(wv_sb, wv_d[:, :, c0:c1])

                    for ci, (n0, sz) in enumerate(chunks):
                        first, last = ci == 0, ci == len(chunks) - 1
                        xh = p1r.tile([128, NK, 128], f16, tag="xh",
                                      name="xh")
                        nc.sync.dma_start(xh[:, :, :sz],
                                          xT_r[:, :, n0:n0 + sz])

                        pk = p1ps.tile([128, 384], f32, tag="pk", name="pk")
                        pv = p1ps.tile([128, 384], f32, tag="pv", name="pv")
                        for k in range(NK):
                            nc.tensor.matmul(pv[:sz], xh[:, k, :sz],
                                             wv_sb[:, k, :], start=(k == 0),
                                             stop=(k == NK - 1))
                        for k in range(NK):
                            nc.tensor.matmul(pk[:sz], xh[:, k, :sz],
                                             wk_sb[:, k, :], start=(k == 0),
                                             stop=(k == NK - 1))
                        v_sb = p1r.tile([128, 384], f16, tag="v",
                                        name="v_sb")
                        nc.scalar.copy(v_sb[:sz], pv[:sz])
                        e_sb = p1r.tile([128, 384], f16, tag="e",
                                        name="e_sb")
                        nc.scalar.activation(e_sb[:sz], pk[:sz], AF.Exp)

                        for hi, h in enumerate(heads):
                            kvp = kv_ps[h // 4]
                            nc.tensor.matmul(
                                kvp[:, h % 4, 0:CH],
                                e_sb[:sz, hi * CH:(hi + 1) * CH],
                                v_sb[:sz, hi * CH:(hi + 1) * CH],
                                start=first, stop=last,
                                skip_group_check=True)
                            nc.tensor.matmul(
                                kvp[:, h % 4, CH:CH + 1],
                                e_sb[:sz, hi * CH:(hi + 1) * CH],
                                ones[:sz],
                                start=first, stop=last,
                                skip_group_check=True)

                        if first:
                            continue
                        for tj, t in enumerate(tiles):
                            tp = p1ps.tile([128, 128], f16, tag="tp",
                                           name="tp")
                            nc.tensor.transpose(
                                tp[:, :sz],
                                v_sb[:sz, tj * 128:(tj + 1) * 128],
                                ident[:sz, :sz])
                            if hp == 1:
                                nc.scalar.activation(
                                    vt6[:, t, n0 - 1:n0 - 1 + sz],
                                    tp[:, :sz], AF.Identity,
                                    bias=bv_sb[:, t:t + 1])
                                continue
                            # hp == 0: write padded layouts directly
                            y = 1 + 2 * (ci - 1)   # dst row (pitch-64)
                            t2 = tp[:, :sz].rearrange("p (a b) -> p a b",
                                                      b=56)
                            nc.scalar.activation(
                                vtp[t][:, y:y + 2, 0:56], t2, AF.Identity,
                                bias=bv_sb[:, t:t + 1])
                            if t in DVE_TILES:
                                nc.scalar.activation(
                                    vtpo[t][:, y:y + 2, 0:55],
                                    t2[:, :, 1:56], AF.Identity,
                                    bias=bv_sb[:, t:t + 1])
                                nc.scalar.activation(
                                    vtm[t][:, y:y + 2, 1:57], t2,
                                    AF.Identity, bias=bv_sb[:, t:t + 1])

            # zero the pad regions pass1a's direct writes won't touch
            for t in PAD_TILES:
                nc.vector.memset(vtp[t][:, 0:1, :], 0.0)
                nc.vector.memset(vtp[t][:, 1:57, 56:64], 0.0)
                if t in DVE_TILES:
                    nc.vector.memset(vtpo[t][:, 0:1, :], 0.0)
                    nc.vector.memset(vtpo[t][:, 1:57, 55:64], 0.0)
                    nc.vector.memset(vtm[t][:, 0:1, :], 0.0)
                    nc.vector.memset(vtm[t][:, 1:57, 0:1], 0.0)
                    nc.vector.memset(vtm[t][:, 1:57, 57:64], 0.0)
                    if pe_taps(t):
                        nc.gpsimd.memset(acc6[t], 0.0)

            # ---- pass 1a (tiles 3-5, padded v^T layout)
            pass1(0)

            # big weight loads deferred past pass1a's DMAs (first readers
            # are the group loop / cls path)
            nc.sync.dma_start(dg_sb, dg_d[:, :, :])
            nc.sync.dma_start(wq_sb, wq_d[:, :, :])
            nc.sync.dma_start(pw_sb, pw_d[:, :, :])

            b0ops = band_ops(0, 32)
            emit_conv(b0ops[:62], act_budget=0)

            # ---- pass 1b (tiles 0-2)
            pass1(1)

            # evacuate raw kv+den so the PSUM pool frees without waiting
            # on the DVE queue (which is deep in conv work here)
            kvraw = const.tile([CH, 2, 4, CH + 1], f32)
            nc.scalar.copy(kvraw[:, 0], kv_ps[0])
            nc.scalar.copy(kvraw[:, 1], kv_ps[1])

        # ---- kv finalize (DVE, right after pass1b in queue order)
        for h in range(NH):
            kvr = kvraw[:, h // 4]
            nc.vector.reciprocal(r_sb[:, h:h + 1], kvr[:, h % 4, 96:97])
            nc.vector.tensor_scalar(kv_sb[:, h, :], kvr[:, h % 4, 0:CH],
                                    r_sb[:, h:h + 1], float(SCALE),
                                    op0=AL.mult, op1=AL.mult)
        nc.vector.tensor_tensor(
            kv_sb.rearrange("p a b -> p (a b)"),
            kv_sb.rearrange("p a b -> p (a b)"),
            bkv_sb, op=AL.add)
        emit_conv(b0ops[62:], act_budget=0)

        for h in range(NH):
            r0, r1 = 96 * h, 96 * h + 96
            for tk in range(r0 // 128, (r1 - 1) // 128 + 1):
                rr0, rr1 = max(r0, 128 * tk), min(r1, 128 * tk + 128)
                for t in range(r0 // 128, (r1 - 1) // 128 + 1):
                    cc0, cc1 = max(r0, 128 * t), min(r1, 128 * t + 128)
                    nc.sync.dma_start(
                        kv6[rr0 - 128 * tk:rr1 - 128 * tk, KVIDX[(tk, t)],
                            cc0 - 128 * t:cc1 - 128 * t],
                        kv_sb[rr0 - r0:rr1 - r0, h, cc0 - r0:cc1 - r0])

        # ---- group loop + cls
        with tc.tile_pool(name="p2r", bufs=2) as p2r, \
             tc.tile_pool(name="p2cv", bufs=3) as p2cv, \
             tc.tile_pool(name="p2qa", bufs=2) as p2qa, \
             tc.tile_pool(name="p2xg", bufs=2) as p2xg, \
             tc.tile_pool(name="p2ps", bufs=2, space="PSUM") as p2ps:

            xc = p2xg.tile([128, NK, GC], f16, tag="xg", name="xc")
            nc.sync.dma_start(xc[:, :, 0:1], xT_r[:, :, 0:1])
            pqc = p2ps.tile([128, NT], f32, tag="pq", name="pqc")
            for to in range(NT):
                for k in range(NK):
                    nc.tensor.matmul(pqc[:, to:to + 1],
                                     wq_sb[:, k, 128 * to:128 * to + 128],
                                     xc[:, k, 0:1], start=(k == 0),
                                     stop=(k == NK - 1),
                                     skip_group_check=True)
            qtc = p2r.tile([128, NT], f16, tag="qtc", name="qtc")
            for to in range(NT):
                nc.scalar.activation(qtc[:, to:to + 1], pqc[:, to:to + 1],
                                     AF.Identity, bias=bq_sb[:, to:to + 1])
            pfc = p2ps.tile([128, NT], f32, tag="pf", name="pfc")
            for t in range(NT):
                blks = [tk for (tk, tt) in KVBLOCKS if tt == t]
                for bi, tk in enumerate(blks):
                    nc.tensor.matmul(pfc[:, t:t + 1],
                                     kv6[:, KVIDX[(tk, t)], :],
                                     qtc[:, tk:tk + 1], start=(bi == 0),
                                     stop=(bi == len(blks) - 1),
                                     skip_group_check=True)
            atc = p2r.tile([128, NT], f16, tag="atc", name="atc")
            nc.scalar.copy(atc, pfc)
            poc = p2ps.tile([128, NT], f32, tag="po", name="poc")
            for eo in range(NT):
                for tf in range(NT):
                    nc.tensor.matmul(poc[:, eo:eo + 1],
                                     pw_sb[:, tf, 128 * eo:128 * eo + 128],
                                     atc[:, tf:tf + 1], start=(tf == 0),
                                     stop=(tf == NT - 1),
                                     skip_group_check=True)
            ocs = p2r.tile([128, NT], f32, tag="ocs", name="ocs")
            for eo in range(NT):
                nc.scalar.activation(ocs[:, eo:eo + 1], poc[:, eo:eo + 1],
                                     AF.Identity, bias=pb_sb[:, eo:eo + 1])
                nc.sync.dma_start(out_d[128 * eo:128 * eo + 128, 0:1],
                                  ocs[:, eo:eo + 1])

            def group(g):
                gy0, gy1 = g * 8, g * 8 + 8
                n0 = 1 + g * GC
                xg = p2xg.tile([128, NK, GC], f16, tag="xg", name="xg")
                nc.sync.dma_start(xg, xT_r[:, :, n0:n0 + GC])

                qt6 = p2qa.tile([128, NT, GC], f16, tag="qt", name="qt6")
                for to in range(NT):
                    pq = p2ps.tile([128, GC], f32, tag="pq", name="pq")
                    for k in range(NK):
                        nc.tensor.matmul(
                            pq, wq_sb[:, k, 128 * to:128 * to + 128],
                            xg[:, k, :], start=(k == 0), stop=(k == NK - 1))
                    nc.scalar.activation(qt6[:, to, :], pq, AF.Identity,
                                         bias=bq_sb[:, to:to + 1])

                cv_sb = {}
                for t in range(NT):
                    ptaps = pe_taps(t)
                    if not ptaps:
                        continue
                    pc = p2ps.tile([128, 8, W], f32, tag="pcv", name="pcv")
                    live = []
                    for ti in ptaps:
                        dy, dx = TAPS6[t][ti]
                        y0 = max(gy0, -dy)
                        y1 = min(gy1, H - max(0, dy))
                        if y1 > y0:
                            live.append((ti, dy, dx, y0, y1))
                    assert live[0][0] == 0
                    for li, (ti, dy, dx, y0, y1) in enumerate(live):
                        xa = max(0, -dx)
                        xb = W - max(0, dx)
                        if t <= 2:
                            rhs = vt3[:, t, y0 + dy:y1 + dy,
                                      xa + dx:xb + dx]
                        else:
                            rhs = vtp[t][:, 1 + y0 + dy:1 + y1 + dy,
                                         xa + dx:xb + dx]
                        nc.tensor.matmul(
                            pc[:, y0 - gy0:y1 - gy0, xa:xb],
                            dg_sb[:, PE_COL[(t, ti)], :], rhs,
                            start=(li == 0), stop=(li == len(live) - 1),
                            skip_group_check=True)
                    cv = p2cv.tile([128, GC], f16, tag="cv", name="cv")
                    if dve_taps(t):     # split: cv = (pcv + cb) + acc
                        nc.vector.scalar_tensor_tensor(
                            cv, pc.rearrange("p a b -> p (a b)"),
                            cb_sb[:, t:t + 1],
                            acc6[t][:, g * GC:(g + 1) * GC],
                            op0=AL.add, op1=AL.add)
                    else:
                        nc.scalar.activation(
                            cv, pc.rearrange("p a b -> p (a b)"),
                            AF.Identity, bias=cb_sb[:, t:t + 1])
                    cv_sb[t] = cv

                att6 = p2qa.tile([128, NT, GC], f16, tag="att", name="att6")
                for t in range(NT):
                    pf = p2ps.tile([128, GC], f32, tag="pf", name="pf")
                    blks = [tk for (tk, tt) in KVBLOCKS if tt == t]
                    for bi, tk in enumerate(blks):
                        nc.tensor.matmul(pf, kv6[:, KVIDX[(tk, t)], :],
                                         qt6[:, tk, :], start=(bi == 0),
                                         stop=(bi == len(blks) - 1))
                    ev = p2r.tile([128, GC], f16, tag="ev", name="ev")
                    if t in cv_sb:
                        nc.vector.tensor_tensor(ev, qt6[:, t, :], cv_sb[t],
                                                op=AL.mult)
                    else:
                        nc.vector.tensor_tensor(
                            ev, qt6[:, t, :],
                            acc6[t][:, g * GC:(g + 1) * GC], op=AL.mult)
                    nc.vector.scalar_tensor_tensor(att6[:, t, :], pf, 1.0,
                                                   ev, op0=AL.mult,
                                                   op1=AL.add)

                for eo in range(NT):
                    po = p2ps.tile([128, GC], f32, tag="po", name="po")
                    for tf in range(NT):
                        nc.tensor.matmul(
                            po, pw_sb[:, tf, 128 * eo:128 * eo + 128],
                            att6[:, tf, :], start=(tf == 0),
                            stop=(tf == NT - 1))
                    osb = p2r.tile([128, GC], f32, tag="osb", name="osb")
                    nc.scalar.activation(osb, po, AF.Identity,
                                         bias=pb_sb[:, eo:eo + 1])
                    nc.sync.dma_start(out_d[128 * eo:128 * eo + 128,
                                            n0:n0 + GC], osb)

            for g in range(4):
                group(g)
            emit_conv(band_ops(32, 56), act_budget=40)
            for g in range(4, GROUPS):
                group(g)

    nc.compile()
    return nc


def _get_program():
    global _PROG
    if _PROG is None:
        _PROG = _build_program()
    return _PROG


def _host_prep(x, qkv_w, qkv_b, proj_w, proj_b,
               conv3_w, conv3_b, conv5_w, conv5_b, conv7_w, conv7_b):
    qkv_w = np.asarray(qkv_w, np.float32)
    qkv_b = np.asarray(qkv_b, np.float32)
    proj_w = np.asarray(proj_w, np.float32)
    proj_b = np.asarray(proj_b, np.float32)

    def wslab(w):
        return np.ascontiguousarray(
            w.T.reshape(NK, 128, C).transpose(1, 0, 2)).astype(fp16)

    wq = wslab(qkv_w[0:C])
    wk = wslab(qkv_w[C:2 * C])
    wv = wslab(qkv_w[2 * C:3 * C])
    pw6 = wslab(proj_w)

    bq = qkv_b[0:C]
    bv = qkv_b[2 * C:3 * C]

    conv_w = [np.asarray(conv3_w, np.float32),
              np.asarray(conv5_w, np.float32),
              np.asarray(conv7_w, np.float32)]
    conv_b = [np.asarray(conv3_b, np.float32),
              np.asarray(conv5_b, np.float32),
              np.asarray(conv7_b, np.float32)]
    grp_of_head = [0, 0, 1, 1, 1, 2, 2, 2]
    head_in_grp = [0, 1, 0, 1, 2, 0, 1, 2]
    w6 = np.zeros((C, 7, 7), np.float32)
    cbf = np.zeros(C, np.float32)
    for h in range(NH):
        k = HEAD_KS[h]
        p = k // 2
        gi, hg = grp_of_head[h], head_in_grp[h]
        w6[96 * h:96 * h + 96, 3 - p:3 + p + 1, 3 - p:3 + p + 1] = \
            conv_w[gi][hg * CH:(hg + 1) * CH, 0]
        cbf[96 * h:96 * h + 96] = conv_b[gi][hg * CH:(hg + 1) * CH]

    dg6 = np.zeros((128, NPECOL, 128), np.float32)
    w6s = np.zeros((128, NDVECOL), np.float32)
    for t in range(NT):
        for ti in pe_taps(t):
            dy, dx = TAPS6[t][ti]
            np.fill_diagonal(dg6[:, PE_COL[(t, ti)], :],
                             w6[128 * t:128 * t + 128, dy + 3, dx + 3])
        for ti in dve_taps(t):
            dy, dx = TAPS6[t][ti]
            w6s[:, DVE_COL[(t, ti)]] = w6[128 * t:128 * t + 128,
                                          dy + 3, dx + 3]

    def densecol(v):
        return np.ascontiguousarray(v.reshape(NT, 128).T).astype(np.float32)

    shared = {"wq6": wq, "wk": wk, "wv": wv, "pw6": pw6,
              "dg6": dg6.astype(fp16), "w6s": w6s.astype(np.float32),
              "cb6": densecol(cbf), "bq6": densecol(bq),
              "pb6": densecol(proj_b), "bv6": densecol(bv),
              "bvkv": np.tile(bv.reshape(1, NH, CH),
                              (CH, 1, 1)).reshape(CH, NH * CH).astype(fp16)}

    x = np.asarray(x, np.float32)
    in_maps = []
    for b in range(B):
        m = dict(shared)
        m["xT"] = np.ascontiguousarray(x[b].T).astype(fp16)
        in_maps.append(m)
    return in_maps


def kernel(x, qkv_w, qkv_b, proj_w, proj_b,
           conv3_w, conv3_b, conv5_w, conv5_b, conv7_w, conv7_b, H, W,
           _trace=False):
    assert int(H) == 56 and int(W) == 56
    x = np.asarray(x)
    assert x.shape == (B, N, C)

    from concourse.bass_utils import run_bass_kernel_spmd
    nc = _get_program()
    in_maps = _host_prep(x, qkv_w, qkv_b, proj_w, proj_b,
                         conv3_w, conv3_b, conv5_w, conv5_b, conv7_w, conv7_b)
    res = run_bass_kernel_spmd(nc, in_maps, core_ids=list(range(B)),
                               trace=_trace)
    out = np.stack([res.results[b]["outT"].T for b in range(B)])
    if _trace:
        kernel._last_results = res
    return out.astype(np.float32)


# revision 3
# speedup vs baseline: 1.0306x; 1.0038x over previous
"""CoaT factorized-attention block kernel for Trainium2, 8 NeuronCores.

Data-parallel over batch B=8 -> one batch element per core. All-fp16
operands (fp32 PSUM accumulation); dense 128-feature tile layout.

Per-core pipeline:
  pass1a (features 384:768 = heads 4-7, tiles 3-5) per 128-token chunk:
    k,v = x @ Wk/Wv (PE); E = exp(k) (ACT; k-bias cancels in the
    token-axis softmax); kv_aug[h] += E_h^T @ [v_h | 1] (PE, PSUM);
    v^T via PE transpose -> vt tiles (ACT copies, +v-bias).
  padded copies vtp/vtpo/vtm (DMA) for shift-FMA conv reads.
  DVE conv band 0 (rows 0:32) -- overlaps pass1b.
  pass1b: features 0:384 (heads 0-3, tiles 0-2); copies on ACT.
  kv finalize (DVE) -> KV6 block-diag stationary tiles (DMA).
  groups g=0..6 (448 tokens): q dense (PE) -> qt6 (ACT); PE conv taps
    (full-array diagonal matmuls); factor via KV6 (PE); ev = qt*cv,
    att = ev + factor (DVE); proj (PE) -> bias copy (ACT) -> DMA.
    DVE band 1 (rows 32:56) issued after group 3.
  cls token: dedicated tiny q/factor/proj path (crpe = 0).
"""
import numpy as np

B, N, C = 8, 3137, 768
NH, CH = 8, 96
H = W = 56
HW = H * W
NK = 6
NT = 6
GROUPS, GC = 7, 448
SCALE = CH ** -0.5
HEAD_KS = [3, 3, 5, 5, 5, 7, 7, 7]
TILE_KMAX = [3, 5, 5, 7, 7, 7]
fp16 = np.float16


def _taps(k):
    p = k // 2
    return [(0, 0)] + [(dy, dx) for dy in range(-p, p + 1)
                       for dx in range(-p, p + 1) if (dy, dx) != (0, 0)]


TAPS6 = [_taps(k) for k in TILE_KMAX]

# ownership: PE gets the first PE_CNT[t] taps of each tile (full-array
# diagonal matmuls); the rest run as mult(ACT or DVE) + add(DVE) over
# padded shift-copies vtp/vtpo/vtm.
PE_CNT = [9, 25, 25, 49, 12, 0]      # groups 0-3
LATE_CNT = [9, 25, 25, 49, 0, 0]     # groups 4-6: t4 fully off-PE
PAD_TILES = [3, 4, 5]     # tiles whose conv reads padded copies


def pe_taps(t):
    return list(range(PE_CNT[t]))


def dve_taps(t):
    return list(range(PE_CNT[t], len(TAPS6[t])))


def off_cols(t):
    return list(range(min(PE_CNT[t], LATE_CNT[t]), len(TAPS6[t])))


DVE_TILES = [t for t in range(NT) if dve_taps(t)]


PE_COL = {}
_c = 0
for _t in range(NT):
    for _ti in pe_taps(_t):
        PE_COL[(_t, _ti)] = _c
        _c += 1
NPECOL = _c
DVE_COL = {}
_c = 0
for _t in range(NT):
    for _ti in off_cols(_t):
        DVE_COL[(_t, _ti)] = _c
        _c += 1
NDVECOL = _c

KVBLOCKS = []
for _h in range(NH):
    _r0, _r1 = 96 * _h, 96 * _h + 96
    for _tk in range(_r0 // 128, (_r1 - 1) // 128 + 1):
        for _t in range(_r0 // 128, (_r1 - 1) // 128 + 1):
            if (_tk, _t) not in KVBLOCKS:
                KVBLOCKS.append((_tk, _t))
KVIDX = {blk: i for i, blk in enumerate(KVBLOCKS)}
NKVB = len(KVBLOCKS)

HALVES = [(384, 768, range(4, 8), range(3, 6)),
          (0, 384, range(0, 4), range(0, 3))]
# pass1a: 112-token chunks (= 2 image rows) so v^T lands directly in the
# padded row-pitch-64 layout; pass1b: 128-token chunks (flat vt6).
CHUNKS_A = [(0, 1)] + [(1 + 112 * t, 112) for t in range(28)]
CHUNKS_B = [(0, 1)] + [(1 + 128 * t, 128) for t in range(24)] + [(3073, 64)]

_PROG = None


def _build_program():
    from concourse import bacc
    import concourse.mybir as mybir
    import concourse.tile as tile
    from concourse.masks import make_identity
    from contextlib import ExitStack

    f32 = mybir.dt.float32
    f16 = mybir.dt.float16
    AL = mybir.AluOpType
    AF = mybir.ActivationFunctionType

    nc = bacc.Bacc("TRN2", target_bir_lowering=False, debug=False,
                   num_devices=8)

    xT_d = nc.dram_tensor("xT", [C, N], f16, kind="ExternalInput")
    wk_d = nc.dram_tensor("wk", [128, NK, C], f16, kind="ExternalInput")
    wv_d = nc.dram_tensor("wv", [128, NK, C], f16, kind="ExternalInput")
    wq_d = nc.dram_tensor("wq6", [128, NK, C], f16, kind="ExternalInput")
    pw_d = nc.dram_tensor("pw6", [128, NT, C], f16, kind="ExternalInput")
    dg_d = nc.dram_tensor("dg6", [128, NPECOL, 128], f16,
                          kind="ExternalInput")
    ws_d = nc.dram_tensor("w6s", [128, NDVECOL], f32, kind="ExternalInput")
    cb_d = nc.dram_tensor("cb6", [128, NT], f32, kind="ExternalInput")
    bq_d = nc.dram_tensor("bq6", [128, NT], f32, kind="ExternalInput")
    pb_d = nc.dram_tensor("pb6", [128, NT], f32, kind="ExternalInput")
    bv_d = nc.dram_tensor("bv6", [128, NT], f32, kind="ExternalInput")
    bkv_d = nc.dram_tensor("bvkv", [CH, NH * CH], f16, kind="ExternalInput")
    out_d = nc.dram_tensor("outT", [C, N], f32, kind="ExternalOutput")

    xT_r = xT_d[:, :].rearrange("(t p) n -> p t n", p=128)

    with tile.TileContext(nc) as tc, ExitStack() as ctx:
        const = ctx.enter_context(tc.tile_pool(name="const", bufs=1))
        wq_sb = const.tile([128, NK, C], f16)
        pw_sb = const.tile([128, NT, C], f16)
        dg_sb = const.tile([128, NPECOL, 128], f16)
        ws_sb = const.tile([128, NDVECOL], f32)
        nc.sync.dma_start(ws_sb, ws_d[:, :])
        cb_sb = const.tile([128, NT], f32)
        nc.sync.dma_start(cb_sb, cb_d[:, :])
        bq_sb = const.tile([128, NT], f32)
        nc.sync.dma_start(bq_sb, bq_d[:, :])
        pb_sb = const.tile([128, NT], f32)
        nc.sync.dma_start(pb_sb, pb_d[:, :])
        bv_sb = const.tile([128, NT], f32)
        nc.sync.dma_start(bv_sb, bv_d[:, :])
        bkv_sb = const.tile([CH, NH * CH], f16)
        nc.sync.dma_start(bkv_sb, bkv_d[:, :])
        ident = const.tile([128, 128], f16)
        make_identity(nc, ident)
        ones = const.tile([128, 1], f16)
        nc.vector.memset(ones, 1.0)

        # v^T for PE-conv tiles 0-2 (written in pass1b, read all groups)
        vt6 = const.tile([128, 3, HW], f16)
        vt3 = vt6.rearrange("p t (y x) -> p t y x", y=H)

        kv_sb = const.tile([CH, NH, CH], f16)
        r_sb = const.tile([CH, NH], f32)
        kv6 = const.tile([128, NKVB, 128], f16)
        for i in range(NKVB):
            nc.vector.memset(kv6[:, i, :], 0.0)

        # padded shift-copies + DVE accumulators
        vtp, vtpo, vtm, acc6 = {}, {}, {}, {}
        for t in PAD_TILES:
            vtp[t] = const.tile([128, 57, 64], f16, tag=f"vtp{t}",
                                name=f"vtp{t}")
        for t in DVE_TILES:
            vtpo[t] = const.tile([128, 57, 64], f16, tag=f"vtpo{t}",
                                 name=f"vtpo{t}")
            vtm[t] = const.tile([128, 57, 64], f16, tag=f"vtm{t}",
                                name=f"vtm{t}")
            acc6[t] = const.tile([128, HW], f16, tag=f"acc{t}",
                                 name=f"acc{t}")
        cvt_pool = ctx.enter_context(tc.tile_pool(name="cvt", bufs=3))

        def _tap_src(t, dy, dx, y0, y1):
            if dx % 2 == 0:
                src, xo = vtp[t], dx
            elif dx > 0:
                src, xo = vtpo[t], dx - 1
            else:
                src, xo = vtm[t], dx + 1
            if xo >= 0:
                return src[:, 1 + y0 + dy:1 + y1 + dy, xo:xo + 56]
            o0 = (1 + y0 + dy) * 64 + xo
            return src.rearrange("p a b -> p (a b)") \
                [:, o0:o0 + (y1 - y0) * 64] \
                .rearrange("p (a b) -> p a b", b=64)[:, :, 0:56]

        def band_ops(b0, b1, cnt=PE_CNT):
            """Off-PE conv op list for rows [b0,b1), tiles interleaved."""
            ops = []
            maxtap = max(len(TAPS6[t]) - cnt[t] for t in DVE_TILES)
            for i in range(maxtap):
                for t in DVE_TILES:
                    taps = list(range(cnt[t], len(TAPS6[t])))
                    if i >= len(taps):
                        continue
                    ti = taps[i]
                    dy, dx = TAPS6[t][ti]
                    y0 = max(b0, -dy)
                    y1 = min(b1, H - max(0, dy))
                    if y1 <= y0:
                        continue
                    ops.append((t, ti, dy, dx, y0, y1,
                                i == 0 and not pe_taps(t)))
            return ops

        def emit_conv(ops, act_budget):
            acc3 = {t: acc6[t].rearrange("p (y x) -> p y x", y=H)
                    for t in DVE_TILES}
            nact = 0
            for (t, ti, dy, dx, y0, y1, seed) in ops:
                sview = _tap_src(t, dy, dx, y0, y1)
                w = ws_sb[:, DVE_COL[(t, ti)]:DVE_COL[(t, ti)] + 1]
                dst = acc3[t][:, y0:y1, :]
                if seed:
                    # tap (0,0): full band coverage -> seed + conv bias
                    nc.vector.tensor_scalar(dst, sview, w,
                                            cb_sb[:, t:t + 1],
                                            op0=AL.mult, op1=AL.add)
                    continue
                tmp = cvt_pool.tile([128, 32, 56], f16, tag="cvt",
                                    name="cvt")
                tv = tmp[:, 0:y1 - y0, :]
                if nact < act_budget:
                    nact += 1
                    nc.scalar.mul(tv, sview, w)
                else:
                    nc.vector.tensor_scalar_mul(tv, sview, w)
                nc.vector.tensor_tensor(dst, dst, tv, op=AL.add)

        with tc.tile_pool(name="kvps", bufs=1, space="PSUM") as kvpsp:
            kv_ps = [kvpsp.tile([CH, 4, CH + 1], f32, tag=f"kv{i}",
                                name=f"kvps{i}") for i in range(2)]

            def pass1(hp):
                c0, c1, heads, tiles = HALVES[hp]
                chunks = CHUNKS_A if hp == 0 else CHUNKS_B
                with tc.tile_pool(name=f"p1w{hp}", bufs=1) as p1w, \
                     tc.tile_pool(name=f"p1r{hp}", bufs=3) as p1r, \
                     tc.tile_pool(name=f"p1ps{hp}", bufs=2,
                                  space="PSUM") as p1ps:
                    wk_sb = p1w.tile([128, NK, 384], f16, name="wk_sb")
                    nc.sync.dma_start(wk_sb, wk_d[:, :, c0:c1])
                    wv_sb = p1w.tile([128, NK, 384], f16, name="wv_sb")
                    nc.sync.dma_start(wv_sb, wv_d[:, :, c0:c1])

                    for ci, (n0, sz) in enumerate(chunks):
                        first, last = ci == 0, ci == len(chunks) - 1
                        xh = p1r.tile([128, NK, 128], f16, tag="xh",
                                      name="xh")
                        nc.sync.dma_start(xh[:, :, :sz],
                                          xT_r[:, :, n0:n0 + sz])

                        pk = p1ps.tile([128, 384], f32, tag="pk", name="pk")
                        pv = p1ps.tile([128, 384], f32, tag="pv", name="pv")
                        for k in range(NK):
                            nc.tensor.matmul(pv[:sz], xh[:, k, :sz],
                                             wv_sb[:, k, :], start=(k == 0),
                                             stop=(k == NK - 1))
                        for k in range(NK):
                            nc.tensor.matmul(pk[:sz], xh[:, k, :sz],
                                             wk_sb[:, k, :], start=(k == 0),
                                             stop=(k == NK - 1))
                        v_sb = p1r.tile([128, 384], f16, tag="v",
                                        name="v_sb")
                        nc.scalar.copy(v_sb[:sz], pv[:sz])
                        e_sb = p1r.tile([128, 384], f16, tag="e",
                                        name="e_sb")
                        nc.scalar.activation(e_sb[:sz], pk[:sz], AF.Exp)

                        for hi, h in enumerate(heads):
                            kvp = kv_ps[h // 4]
                            nc.tensor.matmul(
                                kvp[:, h % 4, 0:CH],
                                e_sb[:sz, hi * CH:(hi + 1) * CH],
                                v_sb[:sz, hi * CH:(hi + 1) * CH],
                                start=first, stop=last,
                                skip_group_check=True)
                            nc.tensor.matmul(
                                kvp[:, h % 4, CH:CH + 1],
                                e_sb[:sz, hi * CH:(hi + 1) * CH],
                                ones[:sz],
                                start=first, stop=last,
                                skip_group_check=True)

                        if first:
                            continue
                        for tj, t in enumerate(tiles):
                            tp = p1ps.tile([128, 128], f16, tag="tp",
                                           name="tp")
                            nc.tensor.transpose(
                                tp[:, :sz],
                                v_sb[:sz, tj * 128:(tj + 1) * 128],
                                ident[:sz, :sz])
                            if hp == 1:
                                nc.scalar.activation(
                                    vt6[:, t, n0 - 1:n0 - 1 + sz],
                                    tp[:, :sz], AF.Identity,
                                    bias=bv_sb[:, t:t + 1])
                                continue
                            # hp == 0: write padded layouts directly
                            y = 1 + 2 * (ci - 1)   # dst row (pitch-64)
                            t2 = tp[:, :sz].rearrange("p (a b) -> p a b",
                                                      b=56)
                            nc.scalar.activation(
                                vtp[t][:, y:y + 2, 0:56], t2, AF.Identity,
                                bias=bv_sb[:, t:t + 1])
                            if t in DVE_TILES:
                                nc.scalar.activation(
                                    vtpo[t][:, y:y + 2, 0:55],
                                    t2[:, :, 1:56], AF.Identity,
                                    bias=bv_sb[:, t:t + 1])
                                nc.scalar.activation(
                                    vtm[t][:, y:y + 2, 1:57], t2,
                                    AF.Identity, bias=bv_sb[:, t:t + 1])

            # zero the pad regions pass1a's direct writes won't touch
            for t in PAD_TILES:
                nc.vector.memset(vtp[t][:, 0:1, :], 0.0)
                nc.vector.memset(vtp[t][:, 1:57, 56:64], 0.0)
                if t in DVE_TILES:
                    nc.vector.memset(vtpo[t][:, 0:1, :], 0.0)
                    nc.vector.memset(vtpo[t][:, 1:57, 55:64], 0.0)
                    nc.vector.memset(vtm[t][:, 0:1, :], 0.0)
                    nc.vector.memset(vtm[t][:, 1:57, 0:1], 0.0)
                    nc.vector.memset(vtm[t][:, 1:57, 57:64], 0.0)
                    if pe_taps(t):
                        nc.gpsimd.memset(acc6[t], 0.0)

            # ---- pass 1a (tiles 3-5, padded v^T layout)
            pass1(0)

            # big weight loads deferred past pass1a's DMAs (first readers
            # are the group loop / cls path)
            nc.sync.dma_start(dg_sb, dg_d[:, :, :])
            nc.sync.dma_start(wq_sb, wq_d[:, :, :])
            nc.sync.dma_start(pw_sb, pw_d[:, :, :])

            b0ops = band_ops(0, 32)
            emit_conv(b0ops[:62], act_budget=0)

            # ---- pass 1b (tiles 0-2)
            pass1(1)

            # evacuate raw kv+den so the PSUM pool frees without waiting
            # on the DVE queue (which is deep in conv work here)
            kvraw = const.tile([CH, 2, 4, CH + 1], f32)
            nc.scalar.copy(kvraw[:, 0], kv_ps[0])
            nc.scalar.copy(kvraw[:, 1], kv_ps[1])

        # ---- kv finalize (DVE, right after pass1b in queue order)
        for h in range(NH):
            kvr = kvraw[:, h // 4]
            nc.vector.reciprocal(r_sb[:, h:h + 1], kvr[:, h % 4, 96:97])
            nc.vector.tensor_scalar(kv_sb[:, h, :], kvr[:, h % 4, 0:CH],
                                    r_sb[:, h:h + 1], float(SCALE),
                                    op0=AL.mult, op1=AL.mult)
        nc.vector.tensor_tensor(
            kv_sb.rearrange("p a b -> p (a b)"),
            kv_sb.rearrange("p a b -> p (a b)"),
            bkv_sb, op=AL.add)
        emit_conv(b0ops[62:], act_budget=0)

        for h in range(NH):
            r0, r1 = 96 * h, 96 * h + 96
            for tk in range(r0 // 128, (r1 - 1) // 128 + 1):
                rr0, rr1 = max(r0, 128 * tk), min(r1, 128 * tk + 128)
                for t in range(r0 // 128, (r1 - 1) // 128 + 1):
                    cc0, cc1 = max(r0, 128 * t), min(r1, 128 * t + 128)
                    nc.sync.dma_start(
                        kv6[rr0 - 128 * tk:rr1 - 128 * tk, KVIDX[(tk, t)],
                            cc0 - 128 * t:cc1 - 128 * t],
                        kv_sb[rr0 - r0:rr1 - r0, h, cc0 - r0:cc1 - r0])

        # ---- group loop + cls
        with tc.tile_pool(name="p2r", bufs=2) as p2r, \
             tc.tile_pool(name="p2cv", bufs=3) as p2cv, \
             tc.tile_pool(name="p2qa", bufs=2) as p2qa, \
             tc.tile_pool(name="p2xg", bufs=2) as p2xg, \
             tc.tile_pool(name="p2ps", bufs=2, space="PSUM") as p2ps:

            xc = p2xg.tile([128, NK, GC], f16, tag="xg", name="xc")
            nc.sync.dma_start(xc[:, :, 0:1], xT_r[:, :, 0:1])
            pqc = p2ps.tile([128, NT], f32, tag="pq", name="pqc")
            for to in range(NT):
                for k in range(NK):
                    nc.tensor.matmul(pqc[:, to:to + 1],
                                     wq_sb[:, k, 128 * to:128 * to + 128],
                                     xc[:, k, 0:1], start=(k == 0),
                                     stop=(k == NK - 1),
                                     skip_group_check=True)
            qtc = p2r.tile([128, NT], f16, tag="qtc", name="qtc")
            for to in range(NT):
                nc.scalar.activation(qtc[:, to:to + 1], pqc[:, to:to + 1],
                                     AF.Identity, bias=bq_sb[:, to:to + 1])
            pfc = p2ps.tile([128, NT], f32, tag="pf", name="pfc")
            for t in range(NT):
                blks = [tk for (tk, tt) in KVBLOCKS if tt == t]
                for bi, tk in enumerate(blks):
                    nc.tensor.matmul(pfc[:, t:t + 1],
                                     kv6[:, KVIDX[(tk, t)], :],
                                     qtc[:, tk:tk + 1], start=(bi == 0),
                                     stop=(bi == len(blks) - 1),
                                     skip_group_check=True)
            atc = p2r.tile([128, NT], f16, tag="atc", name="atc")
            nc.scalar.copy(atc, pfc)
            poc = p2ps.tile([128, NT], f32, tag="po", name="poc")
            for eo in range(NT):
                for tf in range(NT):
                    nc.tensor.matmul(poc[:, eo:eo + 1],
                                     pw_sb[:, tf, 128 * eo:128 * eo + 128],
                                     atc[:, tf:tf + 1], start=(tf == 0),
                                     stop=(tf == NT - 1),
                                     skip_group_check=True)
            ocs = p2r.tile([128, NT], f32, tag="ocs", name="ocs")
            for eo in range(NT):
                nc.scalar.activation(ocs[:, eo:eo + 1], poc[:, eo:eo + 1],
                                     AF.Identity, bias=pb_sb[:, eo:eo + 1])
                nc.sync.dma_start(out_d[128 * eo:128 * eo + 128, 0:1],
                                  ocs[:, eo:eo + 1])

            def group(g):
                gy0, gy1 = g * 8, g * 8 + 8
                n0 = 1 + g * GC
                xg = p2xg.tile([128, NK, GC], f16, tag="xg", name="xg")
                nc.sync.dma_start(xg, xT_r[:, :, n0:n0 + GC])

                qt6 = p2qa.tile([128, NT, GC], f16, tag="qt", name="qt6")
                for to in range(NT):
                    pq = p2ps.tile([128, GC], f32, tag="pq", name="pq")
                    for k in range(NK):
                        nc.tensor.matmul(
                            pq, wq_sb[:, k, 128 * to:128 * to + 128],
                            xg[:, k, :], start=(k == 0), stop=(k == NK - 1))
                    nc.scalar.activation(qt6[:, to, :], pq, AF.Identity,
                                         bias=bq_sb[:, to:to + 1])

                cv_sb = {}
                gcnt = PE_CNT if g < 4 else LATE_CNT
                for t in range(NT):
                    ptaps = list(range(gcnt[t]))
                    if not ptaps:
                        continue
                    pc = p2ps.tile([128, 8, W], f32, tag="pcv", name="pcv")
                    live = []
                    for ti in ptaps:
                        dy, dx = TAPS6[t][ti]
                        y0 = max(gy0, -dy)
                        y1 = min(gy1, H - max(0, dy))
                        if y1 > y0:
                            live.append((ti, dy, dx, y0, y1))
                    assert live[0][0] == 0
                    for li, (ti, dy, dx, y0, y1) in enumerate(live):
                        xa = max(0, -dx)
                        xb = W - max(0, dx)
                        if t <= 2:
                            rhs = vt3[:, t, y0 + dy:y1 + dy,
                                      xa + dx:xb + dx]
                        else:
                            rhs = vtp[t][:, 1 + y0 + dy:1 + y1 + dy,
                                         xa + dx:xb + dx]
                        nc.tensor.matmul(
                            pc[:, y0 - gy0:y1 - gy0, xa:xb],
                            dg_sb[:, PE_COL[(t, ti)], :], rhs,
                            start=(li == 0), stop=(li == len(live) - 1),
                            skip_group_check=True)
                    cv = p2cv.tile([128, GC], f16, tag="cv", name="cv")
                    if dve_taps(t):     # split: cv = (pcv + cb) + acc
                        nc.vector.scalar_tensor_tensor(
                            cv, pc.rearrange("p a b -> p (a b)"),
                            cb_sb[:, t:t + 1],
                            acc6[t][:, g * GC:(g + 1) * GC],
                            op0=AL.add, op1=AL.add)
                    else:
                        nc.scalar.activation(
                            cv, pc.rearrange("p a b -> p (a b)"),
                            AF.Identity, bias=cb_sb[:, t:t + 1])
                    cv_sb[t] = cv

                att6 = p2qa.tile([128, NT, GC], f16, tag="att", name="att6")
                for t in range(NT):
                    pf = p2ps.tile([128, GC], f32, tag="pf", name="pf")
                    blks = [tk for (tk, tt) in KVBLOCKS if tt == t]
                    for bi, tk in enumerate(blks):
                        nc.tensor.matmul(pf, kv6[:, KVIDX[(tk, t)], :],
                                         qt6[:, tk, :], start=(bi == 0),
                                         stop=(bi == len(blks) - 1))
                    ev = p2r.tile([128, GC], f16, tag="ev", name="ev")
                    if t in cv_sb:
                        nc.vector.tensor_tensor(ev, qt6[:, t, :], cv_sb[t],
                                                op=AL.mult)
                    else:
                        nc.vector.tensor_tensor(
                            ev, qt6[:, t, :],
                            acc6[t][:, g * GC:(g + 1) * GC], op=AL.mult)
                    nc.vector.scalar_tensor_tensor(att6[:, t, :], pf, 1.0,
                                                   ev, op0=AL.mult,
                                                   op1=AL.add)

                for eo in range(NT):
                    po = p2ps.tile([128, GC], f32, tag="po", name="po")
                    for tf in range(NT):
                        nc.tensor.matmul(
                            po, pw_sb[:, tf, 128 * eo:128 * eo + 128],
                            att6[:, tf, :], start=(tf == 0),
                            stop=(tf == NT - 1))
                    osb = p2r.tile([128, GC], f32, tag="osb", name="osb")
                    nc.scalar.activation(osb, po, AF.Identity,
                                         bias=pb_sb[:, eo:eo + 1])
                    nc.sync.dma_start(out_d[128 * eo:128 * eo + 128,
                                            n0:n0 + GC], osb)

            for g in range(4):
                group(g)
            emit_conv(band_ops(32, 56, LATE_CNT), act_budget=48)
            # conv bias for t4's late rows (early rows get it in the merge)
            nc.vector.tensor_scalar_add(acc6[4][:, 32 * W:], acc6[4][:, 32 * W:],
                                        cb_sb[:, 4:5])
            for g in range(4, GROUPS):
                group(g)

    nc.compile()
    return nc


def _get_program():
    global _PROG
    if _PROG is None:
        _PROG = _build_program()
    return _PROG


def _host_prep(x, qkv_w, qkv_b, proj_w, proj_b,
               conv3_w, conv3_b, conv5_w, conv5_b, conv7_w, conv7_b):
    qkv_w = np.asarray(qkv_w, np.float32)
    qkv_b = np.asarray(qkv_b, np.float32)
    proj_w = np.asarray(proj_w, np.float32)
    proj_b = np.asarray(proj_b, np.float32)

    def wslab(w):
        return np.ascontiguousarray(
            w.T.reshape(NK, 128, C).transpose(1, 0, 2)).astype(fp16)

    wq = wslab(qkv_w[0:C])
    wk = wslab(qkv_w[C:2 * C])
    wv = wslab(qkv_w[2 * C:3 * C])
    pw6 = wslab(proj_w)

    bq = qkv_b[0:C]
    bv = qkv_b[2 * C:3 * C]

    conv_w = [np.asarray(conv3_w, np.float32),
              np.asarray(conv5_w, np.float32),
              np.asarray(conv7_w, np.float32)]
    conv_b = [np.asarray(conv3_b, np.float32),
              np.asarray(conv5_b, np.float32),
              np.asarray(conv7_b, np.float32)]
    grp_of_head = [0, 0, 1, 1, 1, 2, 2, 2]
    head_in_grp = [0, 1, 0, 1, 2, 0, 1, 2]
    w6 = np.zeros((C, 7, 7), np.float32)
    cbf = np.zeros(C, np.float32)
    for h in range(NH):
        k = HEAD_KS[h]
        p = k // 2
        gi, hg = grp_of_head[h], head_in_grp[h]
        w6[96 * h:96 * h + 96, 3 - p:3 + p + 1, 3 - p:3 + p + 1] = \
            conv_w[gi][hg * CH:(hg + 1) * CH, 0]
        cbf[96 * h:96 * h + 96] = conv_b[gi][hg * CH:(hg + 1) * CH]

    dg6 = np.zeros((128, NPECOL, 128), np.float32)
    w6s = np.zeros((128, NDVECOL), np.float32)
    for t in range(NT):
        for ti in pe_taps(t):
            dy, dx = TAPS6[t][ti]
            np.fill_diagonal(dg6[:, PE_COL[(t, ti)], :],
                             w6[128 * t:128 * t + 128, dy + 3, dx + 3])
        for ti in off_cols(t):
            dy, dx = TAPS6[t][ti]
            w6s[:, DVE_COL[(t, ti)]] = w6[128 * t:128 * t + 128,
                                          dy + 3, dx + 3]

    def densecol(v):
        return np.ascontiguousarray(v.reshape(NT, 128).T).astype(np.float32)

    shared = {"wq6": wq, "wk": wk, "wv": wv, "pw6": pw6,
              "dg6": dg6.astype(fp16), "w6s": w6s.astype(np.float32),
              "cb6": densecol(cbf), "bq6": densecol(bq),
              "pb6": densecol(proj_b), "bv6": densecol(bv),
              "bvkv": np.tile(bv.reshape(1, NH, CH),
                              (CH, 1, 1)).reshape(CH, NH * CH).astype(fp16)}

    x = np.asarray(x, np.float32)
    in_maps = []
    for b in range(B):
        m = dict(shared)
        m["xT"] = np.ascontiguousarray(x[b].T).astype(fp16)
        in_maps.append(m)
    return in_maps


def kernel(x, qkv_w, qkv_b, proj_w, proj_b,
           conv3_w, conv3_b, conv5_w, conv5_b, conv7_w, conv7_b, H, W,
           _trace=False):
    assert int(H) == 56 and int(W) == 56
    x = np.asarray(x)
    assert x.shape == (B, N, C)

    from concourse.bass_utils import run_bass_kernel_spmd
    nc = _get_program()
    in_maps = _host_prep(x, qkv_w, qkv_b, proj_w, proj_b,
                         conv3_w, conv3_b, conv5_w, conv5_b, conv7_w, conv7_b)
    res = run_bass_kernel_spmd(nc, in_maps, core_ids=list(range(B)),
                               trace=_trace)
    out = np.stack([res.results[b]["outT"].T for b in range(B)])
    if _trace:
        kernel._last_results = res
    return out.astype(np.float32)


# revision 4
# speedup vs baseline: 1.0371x; 1.0063x over previous
"""CoaT factorized-attention block kernel for Trainium2, 8 NeuronCores.

Data-parallel over batch B=8 -> one batch element per core. All-fp16
operands (fp32 PSUM accumulation); dense 128-feature tile layout.

Per-core pipeline:
  pass1a (features 384:768 = heads 4-7, tiles 3-5) per 128-token chunk:
    k,v = x @ Wk/Wv (PE); E = exp(k) (ACT; k-bias cancels in the
    token-axis softmax); kv_aug[h] += E_h^T @ [v_h | 1] (PE, PSUM);
    v^T via PE transpose -> vt tiles (ACT copies, +v-bias).
  padded copies vtp/vtpo/vtm (DMA) for shift-FMA conv reads.
  DVE conv band 0 (rows 0:32) -- overlaps pass1b.
  pass1b: features 0:384 (heads 0-3, tiles 0-2); copies on ACT.
  kv finalize (DVE) -> KV6 block-diag stationary tiles (DMA).
  groups g=0..6 (448 tokens): q dense (PE) -> qt6 (ACT); PE conv taps
    (full-array diagonal matmuls); factor via KV6 (PE); ev = qt*cv,
    att = ev + factor (DVE); proj (PE) -> bias copy (ACT) -> DMA.
    DVE band 1 (rows 32:56) issued after group 3.
  cls token: dedicated tiny q/factor/proj path (crpe = 0).
"""
import numpy as np

B, N, C = 8, 3137, 768
NH, CH = 8, 96
H = W = 56
HW = H * W
NK = 6
NT = 6
GROUPS, GC = 7, 448
SCALE = CH ** -0.5
HEAD_KS = [3, 3, 5, 5, 5, 7, 7, 7]
TILE_KMAX = [3, 5, 5, 7, 7, 7]
fp16 = np.float16


def _taps(k):
    p = k // 2
    return [(0, 0)] + [(dy, dx) for dy in range(-p, p + 1)
                       for dx in range(-p, p + 1) if (dy, dx) != (0, 0)]


TAPS6 = [_taps(k) for k in TILE_KMAX]

# ownership: PE gets the first PE_CNT[t] taps of each tile (full-array
# diagonal matmuls); the rest run as mult(ACT or DVE) + add(DVE) over
# padded shift-copies vtp/vtpo/vtm.
PE_CNT = [9, 25, 25, 49, 12, 0]      # groups 0-3
LATE_CNT = [9, 25, 25, 49, 0, 0]     # groups 4-6: t4 fully off-PE
PAD_TILES = [3, 4, 5]     # tiles whose conv reads padded copies


def pe_taps(t):
    return list(range(PE_CNT[t]))


def dve_taps(t):
    return list(range(PE_CNT[t], len(TAPS6[t])))


def off_cols(t):
    return list(range(min(PE_CNT[t], LATE_CNT[t]), len(TAPS6[t])))


DVE_TILES = [t for t in range(NT) if dve_taps(t)]


PE_COL = {}
_c = 0
for _t in range(NT):
    for _ti in pe_taps(_t):
        PE_COL[(_t, _ti)] = _c
        _c += 1
NPECOL = _c
DVE_COL = {}
_c = 0
for _t in range(NT):
    for _ti in off_cols(_t):
        DVE_COL[(_t, _ti)] = _c
        _c += 1
NDVECOL = _c

KVBLOCKS = []
for _h in range(NH):
    _r0, _r1 = 96 * _h, 96 * _h + 96
    for _tk in range(_r0 // 128, (_r1 - 1) // 128 + 1):
        for _t in range(_r0 // 128, (_r1 - 1) // 128 + 1):
            if (_tk, _t) not in KVBLOCKS:
                KVBLOCKS.append((_tk, _t))
KVIDX = {blk: i for i, blk in enumerate(KVBLOCKS)}
NKVB = len(KVBLOCKS)

HALVES = [(384, 768, range(4, 8), range(3, 6)),
          (0, 384, range(0, 4), range(0, 3))]
# pass1a: 112-token chunks (= 2 image rows) so v^T lands directly in the
# padded row-pitch-64 layout; pass1b: 128-token chunks (flat vt6).
CHUNKS_A = [(0, 1)] + [(1 + 112 * t, 112) for t in range(28)]
CHUNKS_B = [(0, 1)] + [(1 + 128 * t, 128) for t in range(24)] + [(3073, 64)]

_PROG = None


def _build_program():
    from concourse import bacc
    import concourse.mybir as mybir
    import concourse.tile as tile
    from concourse.masks import make_identity
    from contextlib import ExitStack

    f32 = mybir.dt.float32
    f16 = mybir.dt.float16
    AL = mybir.AluOpType
    AF = mybir.ActivationFunctionType

    nc = bacc.Bacc("TRN2", target_bir_lowering=False, debug=False,
                   num_devices=8)

    xT_d = nc.dram_tensor("xT", [C, N], f16, kind="ExternalInput")
    wk_d = nc.dram_tensor("wk", [128, NK, C], f16, kind="ExternalInput")
    wv_d = nc.dram_tensor("wv", [128, NK, C], f16, kind="ExternalInput")
    wq_d = nc.dram_tensor("wq6", [128, NK, C], f16, kind="ExternalInput")
    pw_d = nc.dram_tensor("pw6", [128, NT, C], f16, kind="ExternalInput")
    dg_d = nc.dram_tensor("dg6", [128, NPECOL, 128], f16,
                          kind="ExternalInput")
    ws_d = nc.dram_tensor("w6s", [128, NDVECOL], f32, kind="ExternalInput")
    cb_d = nc.dram_tensor("cb6", [128, NT], f32, kind="ExternalInput")
    bq_d = nc.dram_tensor("bq6", [128, NT], f32, kind="ExternalInput")
    pb_d = nc.dram_tensor("pb6", [128, NT], f32, kind="ExternalInput")
    bv_d = nc.dram_tensor("bv6", [128, NT], f32, kind="ExternalInput")
    bkv_d = nc.dram_tensor("bvkv", [CH, NH * CH], f16, kind="ExternalInput")
    out_d = nc.dram_tensor("outT", [C, N], f32, kind="ExternalOutput")

    xT_r = xT_d[:, :].rearrange("(t p) n -> p t n", p=128)

    with tile.TileContext(nc) as tc, ExitStack() as ctx:
        const = ctx.enter_context(tc.tile_pool(name="const", bufs=1))
        wq_sb = const.tile([128, NK, C], f16)
        pw_sb = const.tile([128, NT, C], f16)
        dg_sb = const.tile([128, NPECOL, 128], f16)
        ws_sb = const.tile([128, NDVECOL], f32)
        nc.sync.dma_start(ws_sb, ws_d[:, :])
        cb_sb = const.tile([128, NT], f32)
        nc.sync.dma_start(cb_sb, cb_d[:, :])
        bq_sb = const.tile([128, NT], f32)
        nc.sync.dma_start(bq_sb, bq_d[:, :])
        pb_sb = const.tile([128, NT], f32)
        nc.sync.dma_start(pb_sb, pb_d[:, :])
        bv_sb = const.tile([128, NT], f32)
        nc.sync.dma_start(bv_sb, bv_d[:, :])
        bkv_sb = const.tile([CH, NH * CH], f16)
        nc.sync.dma_start(bkv_sb, bkv_d[:, :])
        ident = const.tile([128, 128], f16)
        make_identity(nc, ident)
        ones = const.tile([128, 1], f16)
        nc.vector.memset(ones, 1.0)

        # v^T for PE-conv tiles 0-2 (written in pass1b, read all groups)
        vt6 = const.tile([128, 3, HW], f16)
        vt3 = vt6.rearrange("p t (y x) -> p t y x", y=H)

        kv_sb = const.tile([CH, NH, CH], f16)
        r_sb = const.tile([CH, NH], f32)
        kv6 = const.tile([128, NKVB, 128], f16)
        for i in range(NKVB):
            nc.vector.memset(kv6[:, i, :], 0.0)

        # padded shift-copies + DVE accumulators
        vtp, vtpo, vtm, acc6 = {}, {}, {}, {}
        for t in PAD_TILES:
            vtp[t] = const.tile([128, 57, 64], f16, tag=f"vtp{t}",
                                name=f"vtp{t}")
        for t in DVE_TILES:
            vtpo[t] = const.tile([128, 57, 64], f16, tag=f"vtpo{t}",
                                 name=f"vtpo{t}")
            vtm[t] = const.tile([128, 57, 64], f16, tag=f"vtm{t}",
                                name=f"vtm{t}")
            acc6[t] = const.tile([128, HW], f16, tag=f"acc{t}",
                                 name=f"acc{t}")
        cvt_pool = ctx.enter_context(tc.tile_pool(name="cvt", bufs=4))

        def _tap_src(t, dy, dx, y0, y1):
            if dx % 2 == 0:
                src, xo = vtp[t], dx
            elif dx > 0:
                src, xo = vtpo[t], dx - 1
            else:
                src, xo = vtm[t], dx + 1
            if xo >= 0:
                return src[:, 1 + y0 + dy:1 + y1 + dy, xo:xo + 56]
            o0 = (1 + y0 + dy) * 64 + xo
            return src.rearrange("p a b -> p (a b)") \
                [:, o0:o0 + (y1 - y0) * 64] \
                .rearrange("p (a b) -> p a b", b=64)[:, :, 0:56]

        def band_ops(b0, b1, cnt=PE_CNT):
            """Off-PE conv op list for rows [b0,b1), tiles interleaved."""
            ops = []
            maxtap = max(len(TAPS6[t]) - cnt[t] for t in DVE_TILES)
            for i in range(maxtap):
                for t in DVE_TILES:
                    taps = list(range(cnt[t], len(TAPS6[t])))
                    if i >= len(taps):
                        continue
                    ti = taps[i]
                    dy, dx = TAPS6[t][ti]
                    y0 = max(b0, -dy)
                    y1 = min(b1, H - max(0, dy))
                    if y1 <= y0:
                        continue
                    ops.append((t, ti, dy, dx, y0, y1,
                                i == 0 and not pe_taps(t)))
            return ops

        def emit_conv(ops, act_budget):
            acc3 = {t: acc6[t].rearrange("p (y x) -> p y x", y=H)
                    for t in DVE_TILES}
            nact = 0
            for (t, ti, dy, dx, y0, y1, seed) in ops:
                sview = _tap_src(t, dy, dx, y0, y1)
                w = ws_sb[:, DVE_COL[(t, ti)]:DVE_COL[(t, ti)] + 1]
                dst = acc3[t][:, y0:y1, :]
                if seed:
                    # tap (0,0): full band coverage -> seed + conv bias
                    nc.vector.tensor_scalar(dst, sview, w,
                                            cb_sb[:, t:t + 1],
                                            op0=AL.mult, op1=AL.add)
                    continue
                tmp = cvt_pool.tile([128, 32, 56], f16, tag="cvt",
                                    name="cvt")
                tv = tmp[:, 0:y1 - y0, :]
                if nact < act_budget:
                    nact += 1
                    nc.scalar.mul(tv, sview, w)
                else:
                    nc.vector.tensor_scalar_mul(tv, sview, w)
                nc.vector.tensor_tensor(dst, dst, tv, op=AL.add)

        with tc.tile_pool(name="kvps", bufs=1, space="PSUM") as kvpsp:
            kv_ps = [kvpsp.tile([CH, 4, CH + 1], f32, tag=f"kv{i}",
                                name=f"kvps{i}") for i in range(2)]

            def pass1(hp):
                c0, c1, heads, tiles = HALVES[hp]
                chunks = CHUNKS_A if hp == 0 else CHUNKS_B
                with tc.tile_pool(name=f"p1w{hp}", bufs=1) as p1w, \
                     tc.tile_pool(name=f"p1r{hp}", bufs=3) as p1r, \
                     tc.tile_pool(name=f"p1ps{hp}", bufs=2,
                                  space="PSUM") as p1ps:
                    wk_sb = p1w.tile([128, NK, 384], f16, name="wk_sb")
                    wv_sb = p1w.tile([128, NK, 384], f16, name="wv_sb")
                    # split loads so chunk 0's first matmuls start sooner
                    nc.sync.dma_start(wv_sb[:, 0:2, :], wv_d[:, 0:2, c0:c1])
                    nc.sync.dma_start(wk_sb[:, 0:2, :], wk_d[:, 0:2, c0:c1])
                    nc.sync.dma_start(wv_sb[:, 2:6, :], wv_d[:, 2:6, c0:c1])
                    nc.sync.dma_start(wk_sb[:, 2:6, :], wk_d[:, 2:6, c0:c1])

                    for ci, (n0, sz) in enumerate(chunks):
                        first, last = ci == 0, ci == len(chunks) - 1
                        xh = p1r.tile([128, NK, 128], f16, tag="xh",
                                      name="xh")
                        nc.sync.dma_start(xh[:, :, :sz],
                                          xT_r[:, :, n0:n0 + sz])

                        pk = p1ps.tile([128, 384], f32, tag="pk", name="pk")
                        pv = p1ps.tile([128, 384], f32, tag="pv", name="pv")
                        for k in range(NK):
                            nc.tensor.matmul(pv[:sz], xh[:, k, :sz],
                                             wv_sb[:, k, :], start=(k == 0),
                                             stop=(k == NK - 1))
                        for k in range(NK):
                            nc.tensor.matmul(pk[:sz], xh[:, k, :sz],
                                             wk_sb[:, k, :], start=(k == 0),
                                             stop=(k == NK - 1))
                        v_sb = p1r.tile([128, 384], f16, tag="v",
                                        name="v_sb")
                        nc.scalar.copy(v_sb[:sz], pv[:sz])
                        e_sb = p1r.tile([128, 384], f16, tag="e",
                                        name="e_sb")
                        nc.scalar.activation(e_sb[:sz], pk[:sz], AF.Exp)

                        for hi, h in enumerate(heads):
                            kvp = kv_ps[h // 4]
                            nc.tensor.matmul(
                                kvp[:, h % 4, 0:CH],
                                e_sb[:sz, hi * CH:(hi + 1) * CH],
                                v_sb[:sz, hi * CH:(hi + 1) * CH],
                                start=first, stop=last,
                                skip_group_check=True)
                            nc.tensor.matmul(
                                kvp[:, h % 4, CH:CH + 1],
                                e_sb[:sz, hi * CH:(hi + 1) * CH],
                                ones[:sz],
                                start=first, stop=last,
                                skip_group_check=True)

                        if first:
                            continue
                        for tj, t in enumerate(tiles):
                            tp = p1ps.tile([128, 128], f16, tag="tp",
                                           name="tp")
                            nc.tensor.transpose(
                                tp[:, :sz],
                                v_sb[:sz, tj * 128:(tj + 1) * 128],
                                ident[:sz, :sz])
                            if hp == 1:
                                nc.scalar.activation(
                                    vt6[:, t, n0 - 1:n0 - 1 + sz],
                                    tp[:, :sz], AF.Identity,
                                    bias=bv_sb[:, t:t + 1])
                                continue
                            # hp == 0: write padded layouts directly
                            y = 1 + 2 * (ci - 1)   # dst row (pitch-64)
                            t2 = tp[:, :sz].rearrange("p (a b) -> p a b",
                                                      b=56)
                            nc.scalar.activation(
                                vtp[t][:, y:y + 2, 0:56], t2, AF.Identity,
                                bias=bv_sb[:, t:t + 1])
                            if t in DVE_TILES:
                                nc.scalar.activation(
                                    vtpo[t][:, y:y + 2, 0:55],
                                    t2[:, :, 1:56], AF.Identity,
                                    bias=bv_sb[:, t:t + 1])
                                nc.scalar.activation(
                                    vtm[t][:, y:y + 2, 1:57], t2,
                                    AF.Identity, bias=bv_sb[:, t:t + 1])

            # zero the pad regions pass1a's direct writes won't touch
            for t in PAD_TILES:
                nc.vector.memset(vtp[t][:, 0:1, :], 0.0)
                nc.vector.memset(vtp[t][:, 1:57, 56:64], 0.0)
                if t in DVE_TILES:
                    nc.vector.memset(vtpo[t][:, 0:1, :], 0.0)
                    nc.vector.memset(vtpo[t][:, 1:57, 55:64], 0.0)
                    nc.vector.memset(vtm[t][:, 0:1, :], 0.0)
                    nc.vector.memset(vtm[t][:, 1:57, 0:1], 0.0)
                    nc.vector.memset(vtm[t][:, 1:57, 57:64], 0.0)
                    if pe_taps(t):
                        nc.gpsimd.memset(acc6[t], 0.0)

            # ---- pass 1a (tiles 3-5, padded v^T layout)
            pass1(0)

            # big weight loads deferred past pass1a's DMAs (first readers
            # are the group loop / cls path)
            nc.sync.dma_start(dg_sb, dg_d[:, :, :])
            nc.sync.dma_start(wq_sb, wq_d[:, :, :])
            nc.sync.dma_start(pw_sb, pw_d[:, :, :])

            b0ops = band_ops(0, 32)
            emit_conv(b0ops[:62], act_budget=0)

            # ---- pass 1b (tiles 0-2)
            pass1(1)

            # evacuate raw kv+den so the PSUM pool frees without waiting
            # on the DVE queue (which is deep in conv work here)
            kvraw = const.tile([CH, 2, 4, CH + 1], f32)
            nc.scalar.copy(kvraw[:, 0], kv_ps[0])
            nc.scalar.copy(kvraw[:, 1], kv_ps[1])

        # ---- kv finalize (DVE, right after pass1b in queue order)
        for h in range(NH):
            kvr = kvraw[:, h // 4]
            nc.vector.reciprocal(r_sb[:, h:h + 1], kvr[:, h % 4, 96:97])
            nc.vector.tensor_scalar(kv_sb[:, h, :], kvr[:, h % 4, 0:CH],
                                    r_sb[:, h:h + 1], float(SCALE),
                                    op0=AL.mult, op1=AL.mult)
        nc.vector.tensor_tensor(
            kv_sb.rearrange("p a b -> p (a b)"),
            kv_sb.rearrange("p a b -> p (a b)"),
            bkv_sb, op=AL.add)
        emit_conv(b0ops[62:], act_budget=0)

        for h in range(NH):
            r0, r1 = 96 * h, 96 * h + 96
            for tk in range(r0 // 128, (r1 - 1) // 128 + 1):
                rr0, rr1 = max(r0, 128 * tk), min(r1, 128 * tk + 128)
                for t in range(r0 // 128, (r1 - 1) // 128 + 1):
                    cc0, cc1 = max(r0, 128 * t), min(r1, 128 * t + 128)
                    nc.sync.dma_start(
                        kv6[rr0 - 128 * tk:rr1 - 128 * tk, KVIDX[(tk, t)],
                            cc0 - 128 * t:cc1 - 128 * t],
                        kv_sb[rr0 - r0:rr1 - r0, h, cc0 - r0:cc1 - r0])

        # ---- group loop + cls
        with tc.tile_pool(name="p2r", bufs=2) as p2r, \
             tc.tile_pool(name="p2cv", bufs=3) as p2cv, \
             tc.tile_pool(name="p2qa", bufs=2) as p2qa, \
             tc.tile_pool(name="p2xg", bufs=2) as p2xg, \
             tc.tile_pool(name="p2ps", bufs=2, space="PSUM") as p2ps:

            xc = p2xg.tile([128, NK, GC], f16, tag="xg", name="xc")
            nc.sync.dma_start(xc[:, :, 0:1], xT_r[:, :, 0:1])
            pqc = p2ps.tile([128, NT], f32, tag="pq", name="pqc")
            for to in range(NT):
                for k in range(NK):
                    nc.tensor.matmul(pqc[:, to:to + 1],
                                     wq_sb[:, k, 128 * to:128 * to + 128],
                                     xc[:, k, 0:1], start=(k == 0),
                                     stop=(k == NK - 1),
                                     skip_group_check=True)
            qtc = p2r.tile([128, NT], f16, tag="qtc", name="qtc")
            for to in range(NT):
                nc.scalar.activation(qtc[:, to:to + 1], pqc[:, to:to + 1],
                                     AF.Identity, bias=bq_sb[:, to:to + 1])
            pfc = p2ps.tile([128, NT], f32, tag="pf", name="pfc")
            for t in range(NT):
                blks = [tk for (tk, tt) in KVBLOCKS if tt == t]
                for bi, tk in enumerate(blks):
                    nc.tensor.matmul(pfc[:, t:t + 1],
                                     kv6[:, KVIDX[(tk, t)], :],
                                     qtc[:, tk:tk + 1], start=(bi == 0),
                                     stop=(bi == len(blks) - 1),
                                     skip_group_check=True)
            atc = p2r.tile([128, NT], f16, tag="atc", name="atc")
            nc.scalar.copy(atc, pfc)
            poc = p2ps.tile([128, NT], f32, tag="po", name="poc")
            for eo in range(NT):
                for tf in range(NT):
                    nc.tensor.matmul(poc[:, eo:eo + 1],
                                     pw_sb[:, tf, 128 * eo:128 * eo + 128],
                                     atc[:, tf:tf + 1], start=(tf == 0),
                                     stop=(tf == NT - 1),
                                     skip_group_check=True)
            ocs = p2r.tile([128, NT], f32, tag="ocs", name="ocs")
            for eo in range(NT):
                nc.scalar.activation(ocs[:, eo:eo + 1], poc[:, eo:eo + 1],
                                     AF.Identity, bias=pb_sb[:, eo:eo + 1])
                nc.sync.dma_start(out_d[128 * eo:128 * eo + 128, 0:1],
                                  ocs[:, eo:eo + 1])

            def group(g):
                gy0, gy1 = g * 8, g * 8 + 8
                n0 = 1 + g * GC
                xg = p2xg.tile([128, NK, GC], f16, tag="xg", name="xg")
                nc.sync.dma_start(xg, xT_r[:, :, n0:n0 + GC])

                qt6 = p2qa.tile([128, NT, GC], f16, tag="qt", name="qt6")
                for to in range(NT):
                    pq = p2ps.tile([128, GC], f32, tag="pq", name="pq")
                    for k in range(NK):
                        nc.tensor.matmul(
                            pq, wq_sb[:, k, 128 * to:128 * to + 128],
                            xg[:, k, :], start=(k == 0), stop=(k == NK - 1))
                    nc.scalar.activation(qt6[:, to, :], pq, AF.Identity,
                                         bias=bq_sb[:, to:to + 1])

                cv_sb = {}
                gcnt = PE_CNT if g < 4 else LATE_CNT
                for t in range(NT):
                    ptaps = list(range(gcnt[t]))
                    if not ptaps:
                        continue
                    pc = p2ps.tile([128, 8, W], f32, tag="pcv", name="pcv")
                    live = []
                    for ti in ptaps:
                        dy, dx = TAPS6[t][ti]
                        y0 = max(gy0, -dy)
                        y1 = min(gy1, H - max(0, dy))
                        if y1 > y0:
                            live.append((ti, dy, dx, y0, y1))
                    assert live[0][0] == 0
                    for li, (ti, dy, dx, y0, y1) in enumerate(live):
                        xa = max(0, -dx)
                        xb = W - max(0, dx)
                        if t <= 2:
                            rhs = vt3[:, t, y0 + dy:y1 + dy,
                                      xa + dx:xb + dx]
                        else:
                            rhs = vtp[t][:, 1 + y0 + dy:1 + y1 + dy,
                                         xa + dx:xb + dx]
                        nc.tensor.matmul(
                            pc[:, y0 - gy0:y1 - gy0, xa:xb],
                            dg_sb[:, PE_COL[(t, ti)], :], rhs,
                            start=(li == 0), stop=(li == len(live) - 1),
                            skip_group_check=True)
                    cv = p2cv.tile([128, GC], f16, tag="cv", name="cv")
                    if dve_taps(t):     # split: cv = (pcv + cb) + acc
                        nc.vector.scalar_tensor_tensor(
                            cv, pc.rearrange("p a b -> p (a b)"),
                            cb_sb[:, t:t + 1],
                            acc6[t][:, g * GC:(g + 1) * GC],
                            op0=AL.add, op1=AL.add)
                    else:
                        nc.scalar.activation(
                            cv, pc.rearrange("p a b -> p (a b)"),
                            AF.Identity, bias=cb_sb[:, t:t + 1])
                    cv_sb[t] = cv

                att6 = p2qa.tile([128, NT, GC], f16, tag="att", name="att6")
                for t in range(NT):
                    pf = p2ps.tile([128, GC], f32, tag="pf", name="pf")
                    blks = [tk for (tk, tt) in KVBLOCKS if tt == t]
                    for bi, tk in enumerate(blks):
                        nc.tensor.matmul(pf, kv6[:, KVIDX[(tk, t)], :],
                                         qt6[:, tk, :], start=(bi == 0),
                                         stop=(bi == len(blks) - 1))
                    ev = p2r.tile([128, GC], f16, tag="ev", name="ev")
                    if t in cv_sb:
                        nc.vector.tensor_tensor(ev, qt6[:, t, :], cv_sb[t],
                                                op=AL.mult)
                    else:
                        nc.vector.tensor_tensor(
                            ev, qt6[:, t, :],
                            acc6[t][:, g * GC:(g + 1) * GC], op=AL.mult)
                    nc.vector.scalar_tensor_tensor(att6[:, t, :], pf, 1.0,
                                                   ev, op0=AL.mult,
                                                   op1=AL.add)

                for eo in range(NT):
                    po = p2ps.tile([128, GC], f32, tag="po", name="po")
                    for tf in range(NT):
                        nc.tensor.matmul(
                            po, pw_sb[:, tf, 128 * eo:128 * eo + 128],
                            att6[:, tf, :], start=(tf == 0),
                            stop=(tf == NT - 1))
                    osb = p2r.tile([128, GC], f32, tag="osb", name="osb")
                    nc.scalar.activation(osb, po, AF.Identity,
                                         bias=pb_sb[:, eo:eo + 1])
                    nc.sync.dma_start(out_d[128 * eo:128 * eo + 128,
                                            n0:n0 + GC], osb)

            for g in range(4):
                group(g)
            emit_conv(band_ops(32, 56, LATE_CNT), act_budget=56)
            # conv bias for t4's late rows (early rows get it in the merge)
            nc.vector.tensor_scalar_add(acc6[4][:, 32 * W:], acc6[4][:, 32 * W:],
                                        cb_sb[:, 4:5])
            for g in range(4, GROUPS):
                group(g)

    nc.compile()
    return nc


def _get_program():
    global _PROG
    if _PROG is None:
        _PROG = _build_program()
    return _PROG


def _host_prep(x, qkv_w, qkv_b, proj_w, proj_b,
               conv3_w, conv3_b, conv5_w, conv5_b, conv7_w, conv7_b):
    qkv_w = np.asarray(qkv_w, np.float32)
    qkv_b = np.asarray(qkv_b, np.float32)
    proj_w = np.asarray(proj_w, np.float32)
    proj_b = np.asarray(proj_b, np.float32)

    def wslab(w):
        return np.ascontiguousarray(
            w.T.reshape(NK, 128, C).transpose(1, 0, 2)).astype(fp16)

    wq = wslab(qkv_w[0:C])
    wk = wslab(qkv_w[C:2 * C])
    wv = wslab(qkv_w[2 * C:3 * C])
    pw6 = wslab(proj_w)

    bq = qkv_b[0:C]
    bv = qkv_b[2 * C:3 * C]

    conv_w = [np.asarray(conv3_w, np.float32),
              np.asarray(conv5_w, np.float32),
              np.asarray(conv7_w, np.float32)]
    conv_b = [np.asarray(conv3_b, np.float32),
              np.asarray(conv5_b, np.float32),
              np.asarray(conv7_b, np.float32)]
    grp_of_head = [0, 0, 1, 1, 1, 2, 2, 2]
    head_in_grp = [0, 1, 0, 1, 2, 0, 1, 2]
    w6 = np.zeros((C, 7, 7), np.float32)
    cbf = np.zeros(C, np.float32)
    for h in range(NH):
        k = HEAD_KS[h]
        p = k // 2
        gi, hg = grp_of_head[h], head_in_grp[h]
        w6[96 * h:96 * h + 96, 3 - p:3 + p + 1, 3 - p:3 + p + 1] = \
            conv_w[gi][hg * CH:(hg + 1) * CH, 0]
        cbf[96 * h:96 * h + 96] = conv_b[gi][hg * CH:(hg + 1) * CH]

    dg6 = np.zeros((128, NPECOL, 128), np.float32)
    w6s = np.zeros((128, NDVECOL), np.float32)
    for t in range(NT):
        for ti in pe_taps(t):
            dy, dx = TAPS6[t][ti]
            np.fill_diagonal(dg6[:, PE_COL[(t, ti)], :],
                             w6[128 * t:128 * t + 128, dy + 3, dx + 3])
        for ti in off_cols(t):
            dy, dx = TAPS6[t][ti]
            w6s[:, DVE_COL[(t, ti)]] = w6[128 * t:128 * t + 128,
                                          dy + 3, dx + 3]

    def densecol(v):
        return np.ascontiguousarray(v.reshape(NT, 128).T).astype(np.float32)

    shared = {"wq6": wq, "wk": wk, "wv": wv, "pw6": pw6,
              "dg6": dg6.astype(fp16), "w6s": w6s.astype(np.float32),
              "cb6": densecol(cbf), "bq6": densecol(bq),
              "pb6": densecol(proj_b), "bv6": densecol(bv),
              "bvkv": np.tile(bv.reshape(1, NH, CH),
                              (CH, 1, 1)).reshape(CH, NH * CH).astype(fp16)}

    x = np.asarray(x, np.float32)
    in_maps = []
    for b in range(B):
        m = dict(shared)
        m["xT"] = np.ascontiguousarray(x[b].T).astype(fp16)
        in_maps.append(m)
    return in_maps


def kernel(x, qkv_w, qkv_b, proj_w, proj_b,
           conv3_w, conv3_b, conv5_w, conv5_b, conv7_w, conv7_b, H, W,
           _trace=False):
    assert int(H) == 56 and int(W) == 56
    x = np.asarray(x)
    assert x.shape == (B, N, C)

    from concourse.bass_utils import run_bass_kernel_spmd
    nc = _get_program()
    in_maps = _host_prep(x, qkv_w, qkv_b, proj_w, proj_b,
                         conv3_w, conv3_b, conv5_w, conv5_b, conv7_w, conv7_b)
    res = run_bass_kernel_spmd(nc, in_maps, core_ids=list(range(B)),
                               trace=_trace)
    out = np.stack([res.results[b]["outT"].T for b in range(B)])
    if _trace:
        kernel._last_results = res
    return out.astype(np.float32)
